# revision 1
# baseline (speedup 1.0000x reference)
"""EnhancedGTATLayer Trainium2 kernel — 8-core SPMD Bass implementation.

Host: sorts edges by (dst-group, src-half), pads to a uniform cross-core
chunk structure (one SPMD NEFF), packs int16 gather indices and per-slot
(one-hot dst, GCN-norm coefficient) pairs.

Device (per core, 6272 dst nodes = 49 groups of 128), feature-transposed
[feat, node] layout with weight-stationary matmuls:
  phase A: dma_gather x rows by src; S[e,d] = (dstrel[e]==d)*norm[e] built in
           one DVE op; z^T += Xe^T S accumulated in PSUM (one-hot matmuls).
  pass 1:  agg^T = gcn_W^T @ z^T; topo^T = topo_W^T @ x^T; LN stats (sum,
           sum-of-squares) via selector matmuls into per-batch PSUM rows.
  interlude (per 4-chunk batch): rstd = 1/sqrt(var+eps) via DVE Newton
           iteration (no ACT table switches).
  pass 2:  LN normalize + relu (broadcast via selector matmuls), sigmoid
           gates, gated fusion, MLP, residual; PE-transpose to row layout.
"""
import sys

sys.path.insert(0, "/opt/trn_rl_repo")

import numpy as np
import ml_dtypes

BF16 = ml_dtypes.bfloat16

N = 50000
NP = 50176          # padded to 392*128
PC = 6272           # nodes per core = 49*128
NCORES = 8
G = 49              # dst groups of 128 per core
D = 128             # feature dim (CIN == COUT)
H = 4
EPS = 1e-5
HALF = 32768        # int16 index split
GSEG = 1            # dst groups per gather segment
NSEG = 49           # one segment per dst group
BSEG = 4            # dst groups per back-half chunk
NCHUNK = 13         # ceil(49/4)
BATCH = 4           # chunks per stats batch
RSQRT_C = 0x5F3759DF


# ---------------------------------------------------------------- host prep
def _prep(x, edge_index):
    src = np.asarray(edge_index[0], dtype=np.int64)
    dst = np.asarray(edge_index[1], dtype=np.int64)
    loops = np.arange(NP, dtype=np.int64)
    src_all = np.concatenate([src, loops])
    dst_all = np.concatenate([dst, loops])

    deg = np.bincount(dst_all, minlength=NP)
    x_pad = np.zeros((NP, D), dtype=np.float32)
    x_pad[:N] = np.asarray(x, dtype=np.float32)

    core_of = dst_all // PC
    per_core = []
    counts = np.zeros((NCORES, G, 2), dtype=np.int64)
    for c in range(NCORES):
        m = core_of == c
        s = src_all[m]
        dl = dst_all[m] - c * PC
        g = dl >> 7
        h = (s >= HALF).astype(np.int64)
        order = np.lexsort((s, h, g))
        s, dl, h = s[order], dl[order], h[order]
        key = (dl >> 7) * 2 + h
        counts[c] = np.bincount(key, minlength=G * 2).reshape(G, 2)
        per_core.append((s, dl, key))

    cmax = ((counts + 127) // 128).max(axis=0)               # [G, 2] chunks
    ch_off = np.zeros((2, G), dtype=np.int64)
    ch_off[0] = np.concatenate([[0], np.cumsum(cmax[:, 0])[:-1]])
    nch_lo = int(cmax[:, 0].sum())
    ch_off[1] = nch_lo + np.concatenate([[0], np.cumsum(cmax[:, 1])[:-1]])
    totch = nch_lo + int(cmax[:, 1].sum())
    totslots = totch * 128

    dis = deg.astype(np.float32) ** -0.5   # reference: deg ** -0.5 in f32

    idx16_all, dstrel_all, coef_all = [], [], []
    for c in range(NCORES):
        s, dl, key = per_core[c]
        idx = np.zeros(totslots, dtype=np.int16)
        dr = np.full(totslots, -1.0, dtype=np.float32)
        cf = np.zeros(totslots, dtype=np.float32)
        starts = np.concatenate([[0], np.cumsum(np.bincount(key, minlength=G * 2))])
        for g in range(G):
            for h in range(2):
                a, b = starts[g * 2 + h], starts[g * 2 + h + 1]
                if b == a:
                    continue
                off = ch_off[h, g] * 128
                idx[off:off + (b - a)] = (s[a:b] - (HALF if h else 0)).astype(np.int16)
                dr[off:off + (b - a)] = (dl[a:b] & 127).astype(np.float32)
                cf[off:off + (b - a)] = dis[s[a:b]] * dis[dl[a:b] + c * PC]
        idx16_all.append(np.tile(idx.reshape(-1, 16).T, (8, 1)).astype(np.int16))
        dstrel_all.append(np.ascontiguousarray(dr.reshape(totch, 128).T))
        coef_all.append(np.ascontiguousarray(cf.reshape(totch, 128).T))

    meta = dict(cmax=cmax, ch_off=ch_off, totch=totch)
    return x_pad, idx16_all, dstrel_all, coef_all, meta


def _pack_weights(ins):
    w = np.zeros((16, D, D), dtype=np.float32)
    w[0] = ins["gcn_W"]
    w[1] = ins["topo_W"]
    w[2] = ins["res_W"]
    w[3] = ins["mlp_W2"]
    for h in range(H):
        w[4 + h] = ins["mlp_W1"][h * D:(h + 1) * D, :]
        w[8 + 2 * h] = ins["attn_W"][h][:D, :]
        w[9 + 2 * h] = ins["attn_W"][h][D:, :]
    v = np.zeros((D, 13), dtype=np.float32)
    for j, k in enumerate(["gcn_b", "ln_node_g", "ln_node_b", "ln_topo_g",
                           "ln_topo_b", "topo_b"]):
        v[:, j] = ins[k]
    for h in range(H):
        v[:, 6 + h] = ins["attn_b"][h]
    v[:, 10] = ins["mlp_b1"]
    v[:, 11] = ins["mlp_b2"]
    v[:, 12] = ins["res_b"]
    return w, v


# ---------------------------------------------------------------- device
def _build(meta):
    import concourse.bacc as bacc
    import concourse.tile as tile
    from concourse import mybir
    from contextlib import ExitStack

    cmax, ch_off, totch = meta["cmax"], meta["ch_off"], meta["totch"]
    F32, BF, I16, I32 = (mybir.dt.float32, mybir.dt.bfloat16,
                         mybir.dt.int16, mybir.dt.int32)
    AF = mybir.ActivationFunctionType
    OP = mybir.AluOpType

    nc = bacc.Bacc("TRN2", target_bir_lowering=False, num_devices=NCORES,
                   dynamic_dma_scratch_size=65536)
    t_xfull = nc.dram_tensor("xfull", [NP, D], F32, kind="ExternalInput")
    t_xT = nc.dram_tensor("xT", [D, PC], F32, kind="ExternalInput")
    t_idx = nc.dram_tensor("idx16", [128, totch * 8], I16, kind="ExternalInput")
    t_dstrel = nc.dram_tensor("dstrel", [128, totch], F32, kind="ExternalInput")
    t_coef = nc.dram_tensor("coef", [128, totch], F32, kind="ExternalInput")
    t_wpack = nc.dram_tensor("wpack", [16, D, D], F32, kind="ExternalInput")
    t_vpack = nc.dram_tensor("vpack", [D, 13], F32, kind="ExternalInput")
    t_out = nc.dram_tensor("out", [PC, D], F32, kind="ExternalOutput")

    iota_np = np.broadcast_to(np.arange(128, dtype=np.float32), (128, 128))
    t_iota = nc.inline_tensor(iota_np.copy(), name="iota128")
    t_id128 = nc.inline_tensor(np.eye(128, dtype=np.float32), name="ident128")
    idrep8 = np.broadcast_to(np.eye(8, dtype=np.float32), (128, 8, 8)).copy()
    t_idrep8 = nc.inline_tensor(idrep8, name="idrep8")
    sel8 = np.broadcast_to(np.eye(8, dtype=np.float32)[:, :, None], (8, 8, 128))
    t_sel_one = nc.inline_tensor(np.ascontiguousarray(sel8), name="sel_one")
    t_sel_neg = nc.inline_tensor(np.ascontiguousarray(sel8 * (-1.0 / 128.0)),
                                 name="sel_neg")

    # gather segment geometry (uniform across cores)
    seg_lo, seg_hi = [], []
    for s in range(NSEG):
        g0, g1 = s * GSEG, min((s + 1) * GSEG, G)
        seg_lo.append((int(ch_off[0, g0]), int(cmax[g0:g1, 0].sum())))
        seg_hi.append((int(ch_off[1, g0]), int(cmax[g0:g1, 1].sum())))
    max_lo = max(n for _, n in seg_lo)
    max_hi = max(n for _, n in seg_hi)

    with ExitStack() as ctx:
        tc = ctx.enter_context(tile.TileContext(nc))
        keep = ctx.enter_context(tc.tile_pool(name="keep", bufs=1))

        # ---------------- persistent tiles
        wbf = keep.tile([128, 16, D], BF)
        with tc.tile_pool(name="tmpw", bufs=1) as tmpw:
            wp32 = tmpw.tile([128, 16, D], F32)
            nc.sync.dma_start(out=wp32[:], in_=t_wpack.ap().rearrange("b k m -> k b m"))
            nc.scalar.copy(out=wbf[:], in_=wp32[:])
        vp = keep.tile([128, 13], F32)
        nc.sync.dma_start(out=vp[:], in_=t_vpack.ap())
        iota_t = keep.tile([128, 128], F32)
        nc.sync.dma_start(out=iota_t[:], in_=t_iota.ap())
        id128 = keep.tile([128, 128], F32)
        nc.sync.dma_start(out=id128[:], in_=t_id128.ap())
        idrep_t = keep.tile([128, 8, 8], F32)
        nc.sync.dma_start(out=idrep_t[:], in_=t_idrep8.ap())
        sel_one = keep.tile([8, 8, 128], F32)
        nc.sync.dma_start(out=sel_one[:], in_=t_sel_one.ap())
        sel_neg = keep.tile([8, 8, 128], F32)
        nc.sync.dma_start(out=sel_neg[:], in_=t_sel_neg.ap())
        dstrel_sb = keep.tile([128, totch], F32)
        nc.sync.dma_start(out=dstrel_sb[:], in_=t_dstrel.ap())
        coef_sb = keep.tile([128, totch], F32)
        nc.sync.dma_start(out=coef_sb[:], in_=t_coef.ap())
        idx_sb = keep.tile([128, totch * 8], I16)
        nc.sync.dma_start(out=idx_sb[:], in_=t_idx.ap())

        u_all = keep.tile([128, PC], BF)
        xTbf = keep.tile([128, PC], BF)
        # Newton constants
        c_magic = keep.tile([8, 512], I32)
        nc.vector.memset(c_magic[:], RSQRT_C)
        c_one = keep.tile([8, 512], I32)
        nc.vector.memset(c_one[:], 1)

        def vcol(j):
            return vp[:, j:j + 1]

        with ExitStack() as pp:
            sb1 = pp.enter_context(tc.tile_pool(name="sb1", bufs=2))
            sb2 = pp.enter_context(tc.tile_pool(name="sb2", bufs=2))
            nwt = pp.enter_context(tc.tile_pool(name="nwt", bufs=2))
            xel = pp.enter_context(tc.tile_pool(name="xel", bufs=3))
            xeh = pp.enter_context(tc.tile_pool(name="xeh", bufs=3))
            zpsp = pp.enter_context(tc.tile_pool(name="zpsp", bufs=2, space="PSUM"))
            stp = pp.enter_context(tc.tile_pool(name="stp", bufs=1, space="PSUM"))
            lnp = pp.enter_context(tc.tile_pool(name="lnp", bufs=1, space="PSUM"))
            bcp = pp.enter_context(tc.tile_pool(name="bcp", bufs=2, space="PSUM"))
            finp = pp.enter_context(tc.tile_pool(name="finp", bufs=1, space="PSUM"))

            def pass1_chunk(ci, st1, st2, cl):
                g0, g1 = ci * BSEG, min((ci + 1) * BSEG, G)
                W = (g1 - g0) * 128
                n0 = g0 * 128
                segs = sorted({g // GSEG for g in range(g0, g1)})
                xe_map = {}
                for s in segs:
                    lo_start, lo_n = seg_lo[s]
                    hi_start, hi_n = seg_hi[s]
                    xe_lo = xel.tile([128, max_lo, D], F32, tag="xel")
                    xe_hi = xeh.tile([128, max_hi, D], F32, tag="xeh")
                    if lo_n:
                        nc.gpsimd.dma_gather(
                            out_ap=xe_lo[:, :lo_n, :], in_ap=t_xfull.ap()[0:HALF, :],
                            idxs_ap=idx_sb[:, lo_start * 8:(lo_start + lo_n) * 8],
                            num_idxs=lo_n * 128, num_idxs_reg=lo_n * 128,
                            elem_size=D, single_packet=False)
                    if hi_n:
                        nc.gpsimd.dma_gather(
                            out_ap=xe_hi[:, :hi_n, :], in_ap=t_xfull.ap()[HALF:NP, :],
                            idxs_ap=idx_sb[:, hi_start * 8:(hi_start + hi_n) * 8],
                            num_idxs=hi_n * 128, num_idxs_reg=hi_n * 128,
                            elem_size=D, single_packet=False)
                    xe_map[s] = (xe_lo, xe_hi)

                zps = zpsp.tile([128, 512], F32, space="PSUM", tag="zps")
                for g in range(g0, g1):
                    col = (g - g0) * 128
                    xe_lo, xe_hi = xe_map[g // GSEG]
                    mms = []
                    for h, (xe, st) in enumerate(
                            [(xe_lo, seg_lo[g // GSEG][0]),
                             (xe_hi, seg_hi[g // GSEG][0])]):
                        for k in range(int(cmax[g, h])):
                            gch = int(ch_off[h, g]) + k
                            sch = gch - st
                            s_t = sb1.tile([128, 128], F32, tag="s_t")
                            # S[e, d] = (dstrel[e] == d) * norm[e]
                            nc.vector.tensor_scalar(
                                out=s_t[:], in0=iota_t[:],
                                scalar1=dstrel_sb[:, gch:gch + 1],
                                scalar2=coef_sb[:, gch:gch + 1],
                                op0=OP.is_equal, op1=OP.mult)
                            mms.append((xe, sch, s_t))
                    for mi, (xe, sch, s_t) in enumerate(mms):
                        nc.tensor.matmul(
                            out=zps[:, col:col + 128], lhsT=xe[:, sch, :],
                            rhs=s_t[:], start=(mi == 0), stop=(mi == len(mms) - 1))

                # z (norm folded into S) -> bf16 for the agg matmul
                nc.scalar.copy(out=u_all[:, n0:n0 + W], in_=zps[:, :W])

                xt32 = sb1.tile([128, 512], F32, tag="xt32")
                nc.sync.dma_start(out=xt32[:, :W], in_=t_xT.ap()[:, n0:n0 + W])
                nc.scalar.copy(out=xTbf[:, n0:n0 + W], in_=xt32[:, :W])

                # agg/topo matmuls + LN stats rows (s1 rows cl / 4+cl, s2 same)
                aps = zpsp.tile([128, 512], F32, space="PSUM", tag="zps")
                nc.tensor.matmul(out=aps[:, :W], lhsT=wbf[:, 0, :],
                                 rhs=u_all[:, n0:n0 + W], start=True, stop=True)
                y = sb1.tile([128, 512], F32, tag="y")
                sq = sb1.tile([128, 512], F32, tag="sq")
                nc.scalar.activation(out=y[:, :W], in_=aps[:, :W],
                                     func=AF.Identity, bias=vcol(0))
                nc.scalar.activation(out=sq[:, :W], in_=aps[:, :W],
                                     func=AF.Square, bias=vcol(0))
                nc.tensor.matmul(out=st1[:, :W], lhsT=idrep_t[:, cl, :],
                                 rhs=y[:, :W], start=(cl == 0),
                                 stop=(cl == BATCH - 1 or ci == NCHUNK - 1))
                nc.tensor.matmul(out=st2[:, :W], lhsT=idrep_t[:, cl, :],
                                 rhs=sq[:, :W], start=(cl == 0),
                                 stop=(cl == BATCH - 1 or ci == NCHUNK - 1))

                tps = zpsp.tile([128, 512], F32, space="PSUM", tag="zps")
                nc.tensor.matmul(out=tps[:, :W], lhsT=wbf[:, 1, :],
                                 rhs=xTbf[:, n0:n0 + W], start=True, stop=True)
                yt = sb1.tile([128, 512], F32, tag="yt")
                sqt = sb1.tile([128, 512], F32, tag="sqt")
                nc.scalar.activation(out=yt[:, :W], in_=tps[:, :W],
                                     func=AF.Identity, bias=vcol(5))
                nc.scalar.activation(out=sqt[:, :W], in_=tps[:, :W],
                                     func=AF.Square, bias=vcol(5))
                nc.tensor.matmul(out=st1[:, :W], lhsT=idrep_t[:, 4 + cl, :],
                                 rhs=yt[:, :W], start=False,
                                 stop=(cl == BATCH - 1 or ci == NCHUNK - 1),
                                 skip_group_check=True)
                nc.tensor.matmul(out=st2[:, :W], lhsT=idrep_t[:, 4 + cl, :],
                                 rhs=sqt[:, :W], start=False,
                                 stop=(cl == BATCH - 1 or ci == NCHUNK - 1),
                                 skip_group_check=True)

            def interlude(st1, st2):
                """rstd rows [8,512] from per-batch stats via DVE Newton rsqrt."""
                s1 = nwt.tile([8, 512], F32, tag="s1")
                nc.vector.tensor_copy(out=s1[:], in_=st1[:])
                q = nwt.tile([8, 512], F32, tag="q")
                nc.vector.tensor_tensor(out=q[:], in0=s1[:], in1=s1[:], op=OP.mult)
                nc.vector.tensor_scalar(out=q[:], in0=q[:], scalar1=1.0 / 128.0,
                                        scalar2=None, op0=OP.mult)
                v = nwt.tile([8, 512], F32, tag="v")
                nc.vector.tensor_tensor(out=v[:], in0=st2[:], in1=q[:], op=OP.subtract)
                nc.vector.tensor_scalar(out=v[:], in0=v[:], scalar1=1.0 / 128.0,
                                        scalar2=EPS, op0=OP.mult, op1=OP.add)
                # Newton rsqrt: y0 = bits(C - (bits(v) >> 1)); 3 iterations
                yi = nwt.tile([8, 512], I32, tag="yi")
                nc.vector.tensor_tensor(out=yi[:], in0=v[:].bitcast(I32),
                                        in1=c_one[:], op=OP.arith_shift_right)
                nc.vector.tensor_tensor(out=yi[:], in0=c_magic[:], in1=yi[:],
                                        op=OP.subtract)
                y = yi[:].bitcast(F32)
                t = nwt.tile([8, 512], F32, tag="t")
                for _ in range(3):
                    nc.vector.tensor_tensor(out=t[:], in0=y, in1=y, op=OP.mult)
                    nc.vector.tensor_tensor(out=t[:], in0=t[:], in1=v[:], op=OP.mult)
                    nc.vector.tensor_scalar(out=t[:], in0=t[:], scalar1=-0.5,
                                            scalar2=1.5, op0=OP.mult, op1=OP.add)
                    nc.vector.tensor_tensor(out=y, in0=y, in1=t[:], op=OP.mult)
                return s1, yi

            def pass2_chunk(ci, cl, s1rows, rrows):
                g0, g1 = ci * BSEG, min((ci + 1) * BSEG, G)
                W = (g1 - g0) * 128
                n0 = g0 * 128
                rr = rrows[:].bitcast(F32)

                def layernorm(wslot, rhs_all, bias_j, g_j, b_j, ln):
                    ps = lnp.tile([128, 512], F32, space="PSUM", tag="ln")
                    nc.tensor.matmul(out=ps[:, :W], lhsT=wbf[:, wslot, :],
                                     rhs=rhs_all[:, n0:n0 + W], start=True, stop=True)
                    mub = bcp.tile([128, 512], F32, space="PSUM", tag="bc")
                    nc.tensor.matmul(out=mub[:, :W], lhsT=sel_neg[:, ln * 4 + cl, :],
                                     rhs=s1rows[:, :W], start=True, stop=True)
                    rb = bcp.tile([128, 512], F32, space="PSUM", tag="bc")
                    nc.tensor.matmul(out=rb[:, :W], lhsT=sel_one[:, ln * 4 + cl, :],
                                     rhs=rr[:, :W], start=True, stop=True)
                    t1 = sb2.tile([128, 512], F32, tag="t1" + str(ln))
                    nc.vector.tensor_scalar(out=t1[:, :W], in0=ps[:, :W],
                                            scalar1=vcol(bias_j), scalar2=None,
                                            op0=OP.add)
                    nc.vector.tensor_tensor(out=t1[:, :W], in0=t1[:, :W],
                                            in1=mub[:, :W], op=OP.add)
                    nc.vector.tensor_tensor(out=t1[:, :W], in0=t1[:, :W],
                                            in1=rb[:, :W], op=OP.mult)
                    nc.vector.tensor_scalar(out=t1[:, :W], in0=t1[:, :W],
                                            scalar1=vcol(g_j), scalar2=vcol(b_j),
                                            op0=OP.mult, op1=OP.add)
                    o = sb2.tile([128, 512], BF, tag="nf" + str(ln))
                    nc.scalar.activation(out=o[:, :W], in_=t1[:, :W], func=AF.Relu)
                    return o

                nf = layernorm(0, u_all, 0, 1, 2, 0)
                tf = layernorm(1, xTbf, 5, 3, 4, 1)

                diff = sb2.tile([128, 512], BF, tag="diff")
                nc.vector.tensor_tensor(out=diff[:, :W], in0=nf[:, :W],
                                        in1=tf[:, :W], op=OP.subtract)

                h1ps = finp.tile([128, 512], F32, space="PSUM", tag="fin")
                for h in range(H):
                    gps = bcp.tile([128, 512], F32, space="PSUM", tag="bc")
                    nc.tensor.matmul(out=gps[:, :W], lhsT=wbf[:, 8 + 2 * h, :],
                                     rhs=nf[:, :W], start=True, stop=False)
                    nc.tensor.matmul(out=gps[:, :W], lhsT=wbf[:, 9 + 2 * h, :],
                                     rhs=tf[:, :W], start=False, stop=True)
                    gate = sb2.tile([128, 512], BF, tag="gate")
                    nc.scalar.activation(out=gate[:, :W], in_=gps[:, :W],
                                         func=AF.Sigmoid, bias=vcol(6 + h))
                    fh = sb2.tile([128, 512], BF, tag="fh")
                    nc.vector.tensor_tensor(out=fh[:, :W], in0=gate[:, :W],
                                            in1=diff[:, :W], op=OP.mult)
                    nc.vector.tensor_tensor(out=fh[:, :W], in0=fh[:, :W],
                                            in1=tf[:, :W], op=OP.add)
                    nc.tensor.matmul(out=h1ps[:, :W], lhsT=wbf[:, 4 + h, :],
                                     rhs=fh[:, :W], start=(h == 0), stop=(h == H - 1))

                h1 = sb2.tile([128, 512], BF, tag="h1")
                nc.scalar.activation(out=h1[:, :W], in_=h1ps[:, :W],
                                     func=AF.Relu, bias=vcol(10))
                h2ps = finp.tile([128, 512], F32, space="PSUM", tag="fin")
                nc.tensor.matmul(out=h2ps[:, :W], lhsT=wbf[:, 3, :],
                                 rhs=h1[:, :W], start=True, stop=True)
                rps = lnp.tile([128, 512], F32, space="PSUM", tag="ln")
                nc.tensor.matmul(out=rps[:, :W], lhsT=wbf[:, 2, :],
                                 rhs=xTbf[:, n0:n0 + W], start=True, stop=True)
                h2 = sb2.tile([128, 512], F32, tag="h2")
                nc.scalar.activation(out=h2[:, :W], in_=h2ps[:, :W],
                                     func=AF.Relu, bias=vcol(11))
                nc.vector.tensor_tensor(out=h2[:, :W], in0=h2[:, :W],
                                        in1=rps[:, :W], op=OP.add)
                nc.vector.tensor_scalar(out=h2[:, :W], in0=h2[:, :W],
                                        scalar1=vcol(12), scalar2=None, op0=OP.add)

                for g in range(g0, g1):
                    col = (g - g0) * 128
                    otp = finp.tile([128, 512], F32, space="PSUM", tag="fin")
                    nc.tensor.transpose(out=otp[:, :128], in_=h2[:, col:col + 128],
                                        identity=id128[:])
                    orow = sb2.tile([128, 128], F32, tag="orow")
                    nc.scalar.copy(out=orow[:], in_=otp[:, :128])
                    nc.sync.dma_start(out=t_out.ap()[g * 128:(g + 1) * 128, :],
                                      in_=orow[:])

            for b in range((NCHUNK + BATCH - 1) // BATCH):
                c0, c1 = b * BATCH, min((b + 1) * BATCH, NCHUNK)
                st1 = stp.tile([8, 512], F32, space="PSUM", tag="st1")
                st2 = stp.tile([8, 512], F32, space="PSUM", tag="st2")
                for ci in range(c0, c1):
                    pass1_chunk(ci, st1, st2, ci - c0)
                s1rows, rrows = interlude(st1, st2)
                for ci in range(c0, c1):
                    pass2_chunk(ci, ci - c0, s1rows, rrows)

    nc.compile()
    return nc


# ---------------------------------------------------------------- entry
LAST_RESULTS = None
LAST_NC = None
LAST_INMAPS = None


def kernel(**inputs):
    import os
    from concourse.bass_utils import run_bass_kernel_spmd

    x = np.asarray(inputs["x"], dtype=np.float32)
    x_pad, idx16_all, dstrel_all, coef_all, meta = _prep(x, inputs["edge_index"])
    wpack, vpack = _pack_weights({k: np.asarray(v, dtype=np.float32)
                                  for k, v in inputs.items() if k != "edge_index"})

    nc = _build(meta)

    in_maps = []
    for c in range(NCORES):
        xT = np.ascontiguousarray(x_pad[c * PC:(c + 1) * PC].T)
        in_maps.append({
            "xfull": x_pad, "xT": xT,
            "idx16": idx16_all[c], "dstrel": dstrel_all[c], "coef": coef_all[c],
            "wpack": wpack, "vpack": vpack,
        })
    global LAST_RESULTS, LAST_NC, LAST_INMAPS
    LAST_NC, LAST_INMAPS = nc, in_maps
    res = run_bass_kernel_spmd(nc, in_maps, core_ids=list(range(NCORES)),
                               trace=bool(os.environ.get("KTRACE")))
    LAST_RESULTS = res
    out = np.concatenate([res.results[c]["out"] for c in range(NCORES)], axis=0)
    return out[:N].astype(np.float32)



# revision 30
# speedup vs baseline: 1.1880x; 1.1880x over previous
"""EnhancedGTATLayer Trainium2 kernel — 8-core SPMD Bass implementation.

Host: sorts edges by (dst-group, src-half), pads to a uniform cross-core
chunk structure (one SPMD NEFF), packs int16 gather indices and per-slot
(one-hot dst, GCN-norm coefficient) pairs.  x is shipped twice: full copy
in bf16 (gather source) and a per-core feature-transposed slice in bf16.

Device (per core, 6272 dst nodes = 49 groups of 128), feature-transposed
[feat, node] layout, bf16 matmuls throughout (PSUM accumulate fp32):
  pass 1: dma_gather x rows by src (bf16, 256B rows); S[e,d] =
          (dstrel[e]==d)*norm[e] built on DVE in bf16 (4x mode);
          z^T += Xe^T S in PSUM; y0 = agg = gcn_W^T z + b and
          y1 = topo_W^T x + b persisted in bf16; LN stats (sum, sumsq)
          via one-hot selector matmuls into a [32,512] PSUM tile.
  interlude (per 7-chunk batch): R' = rsqrt(128*s2 - s1^2 + 16384*eps)
          (= rstd/128) via DVE Newton; Q[0:16]=s1, Q[16:32]=R' in bf16.
  pass 2: yps = I@y - mu (broadcast matmul accumulated in PSUM);
          ln = Act(yps * R'_bcast, Relu, scale=128*g, bias=beta);
          sigmoid gates; fused = Sum_h W1_h^T(gate_h*diff) + W1s^T topo;
          MLP, residual; PE-transpose to row layout, one DMA per chunk.
"""
import sys

sys.path.insert(0, "/opt/trn_rl_repo")

import numpy as np
import ml_dtypes

BF16 = ml_dtypes.bfloat16

N = 50000
NP = 50176          # padded to 392*128
PC = 6272           # nodes per core = 49*128
PCP = 6656          # padded to 13*512 for uniform 512-wide chunks
NCORES = 8
G = 49              # dst groups of 128 per core
D = 128             # feature dim (CIN == COUT)
H = 4
EPS = 1e-5
HALF = 32768        # int16 index split
GSEG = 2            # dst groups per gather segment
NSEG = 25           # ceil(49/2)
BSEG = 4            # dst groups per chunk (512 cols)
NCHUNK = 13         # ceil(49/4); last chunk has 1 group
BATCHES = [4, 4, 3, 2]  # pipelined chunk batches
RSQRT_C = 0x5F3759DF
NEWTON_ITERS = 1


# ---------------------------------------------------------------- host prep
def _prep(x, edge_index):
    src = np.asarray(edge_index[0], dtype=np.int64)
    dst = np.asarray(edge_index[1], dtype=np.int64)
    loops = np.arange(NP, dtype=np.int64)
    src_all = np.concatenate([src, loops])
    dst_all = np.concatenate([dst, loops])

    deg = np.bincount(dst_all, minlength=NP)
    x_pad = np.zeros((NP, D), dtype=np.float32)
    x_pad[:N] = np.asarray(x, dtype=np.float32)

    core_of = dst_all // PC
    per_core = []
    counts = np.zeros((NCORES, G, 2), dtype=np.int64)
    for c in range(NCORES):
        m = core_of == c
        s = src_all[m]
        dl = dst_all[m] - c * PC
        g = dl >> 7
        h = (s >= HALF).astype(np.int64)
        order = np.lexsort((s, h, g))
        s, dl, h = s[order], dl[order], h[order]
        key = (dl >> 7) * 2 + h
        counts[c] = np.bincount(key, minlength=G * 2).reshape(G, 2)
        per_core.append((s, dl, key))

    cmax = ((counts + 127) // 128).max(axis=0)               # [G, 2] chunks
    ch_off = np.zeros((2, G), dtype=np.int64)
    ch_off[0] = np.concatenate([[0], np.cumsum(cmax[:, 0])[:-1]])
    nch_lo = int(cmax[:, 0].sum())
    ch_off[1] = nch_lo + np.concatenate([[0], np.cumsum(cmax[:, 1])[:-1]])
    totch = nch_lo + int(cmax[:, 1].sum())
    totslots = totch * 128

    dis = deg.astype(np.float32) ** -0.5   # reference: deg ** -0.5 in f32

    idx16_all, dstrel_all, coef_all = [], [], []
    for c in range(NCORES):
        s, dl, key = per_core[c]
        idx = np.zeros(totslots, dtype=np.int16)
        dr = np.full(totslots, -1.0, dtype=np.float32)
        cf = np.zeros(totslots, dtype=np.float32)
        starts = np.concatenate([[0], np.cumsum(np.bincount(key, minlength=G * 2))])
        for g in range(G):
            for h in range(2):
                a, b = starts[g * 2 + h], starts[g * 2 + h + 1]
                if b == a:
                    continue
                off = ch_off[h, g] * 128
                idx[off:off + (b - a)] = (s[a:b] - (HALF if h else 0)).astype(np.int16)
                dr[off:off + (b - a)] = (dl[a:b] & 127).astype(np.float32)
                cf[off:off + (b - a)] = dis[s[a:b]] * dis[dl[a:b] + c * PC]
        idx16_all.append(np.tile(idx.reshape(-1, 16).T, (8, 1)).astype(np.int16))
        dstrel_all.append(np.ascontiguousarray(dr.reshape(totch, 128).T))
        coef_all.append(np.ascontiguousarray(cf.reshape(totch, 128).T))

    meta = dict(cmax=cmax, ch_off=ch_off, totch=totch)
    x_bf = x_pad.astype(BF16)
    return x_pad, x_bf, idx16_all, dstrel_all, coef_all, meta


def _pack_weights(ins):
    w = np.zeros((17, D, D), dtype=np.float32)
    w[0] = ins["gcn_W"]
    w[1] = ins["topo_W"]
    w[2] = ins["res_W"]
    w[3] = ins["mlp_W2"]
    for h in range(H):
        w[4 + h] = ins["mlp_W1"][h * D:(h + 1) * D, :]
        w[8 + 2 * h] = ins["attn_W"][h][:D, :]
        w[9 + 2 * h] = ins["attn_W"][h][D:, :]
        w[16] += ins["mlp_W1"][h * D:(h + 1) * D, :]
    v = np.zeros((D, 13), dtype=np.float32)
    v[:, 0] = ins["gcn_b"]
    v[:, 1] = 128.0 * ins["ln_node_g"]
    v[:, 2] = ins["ln_node_b"]
    v[:, 3] = 128.0 * ins["ln_topo_g"]
    v[:, 4] = ins["ln_topo_b"]
    v[:, 5] = ins["topo_b"]
    for h in range(H):
        v[:, 6 + h] = ins["attn_b"][h]
    v[:, 10] = ins["mlp_b1"]
    v[:, 11] = ins["mlp_b2"]
    v[:, 12] = ins["res_b"]
    return w.astype(BF16), v


# ---------------------------------------------------------------- device
def _build(meta):
    import concourse.bacc as bacc
    import concourse.tile as tile
    from concourse import mybir
    from contextlib import ExitStack

    cmax, ch_off, totch = meta["cmax"], meta["ch_off"], meta["totch"]
    F32, BF, I16, I32 = (mybir.dt.float32, mybir.dt.bfloat16,
                         mybir.dt.int16, mybir.dt.int32)
    AF = mybir.ActivationFunctionType
    OP = mybir.AluOpType

    nc = bacc.Bacc("TRN2", target_bir_lowering=False, num_devices=NCORES,
                   dynamic_dma_scratch_size=65536)
    t_xbf = nc.dram_tensor("xbf", [NP, D], BF, kind="ExternalInput")
    t_xT = nc.dram_tensor("xT", [D, PCP], BF, kind="ExternalInput")
    t_idx = nc.dram_tensor("idx16", [128, totch * 8], I16, kind="ExternalInput")
    t_dstrel = nc.dram_tensor("dstrel", [128, totch], F32, kind="ExternalInput")
    t_coef = nc.dram_tensor("coef", [128, totch], F32, kind="ExternalInput")
    t_wpack = nc.dram_tensor("wpack", [17, D, D], BF, kind="ExternalInput")
    t_vpack = nc.dram_tensor("vpack", [D, 13], F32, kind="ExternalInput")
    t_out = nc.dram_tensor("out", [PC, D], F32, kind="ExternalOutput")

    iota_np = np.broadcast_to(np.arange(128, dtype=np.float32),
                              (128, 128)).astype(BF16)
    t_iota = nc.inline_tensor(iota_np.copy(), name="iota128")
    t_id128 = nc.inline_tensor(np.eye(128, dtype=np.float32).astype(BF16),
                               name="ident128")
    t_id128f = nc.inline_tensor(np.eye(128, dtype=np.float32), name="ident128f")
    # stats selectors: [128, 16, 16], [:, k, j] = (j == k)
    idrep = np.broadcast_to(np.eye(16, dtype=np.float32), (128, 16, 16))
    t_idrep = nc.inline_tensor(idrep.astype(BF16).copy(), name="idrep16")
    # broadcast selectors over the [32, 512] Q tile (rows 0:16 = s1 sums,
    # rows 16:32 = R'):  bqone picks R' row, bqneg adds -s1/128 (= -mu).
    bqone = np.zeros((64, 12, 128), dtype=np.float32)
    bqneg = np.zeros((64, 12, 128), dtype=np.float32)
    for k in range(12):
        bqone[32 + k, k, :] = 1.0
        bqneg[k, k, :] = -1.0 / 128.0
    t_bqone = nc.inline_tensor(bqone.astype(BF16), name="bqone")
    t_bqneg = nc.inline_tensor(bqneg.astype(BF16), name="bqneg")

    # gather segment geometry (uniform across cores)
    seg_lo, seg_hi = [], []
    for s in range(NSEG):
        g0, g1 = s * GSEG, min((s + 1) * GSEG, G)
        seg_lo.append((int(ch_off[0, g0]), int(cmax[g0:g1, 0].sum())))
        seg_hi.append((int(ch_off[1, g0]), int(cmax[g0:g1, 1].sum())))
    max_lo = max(n for _, n in seg_lo)
    max_hi = max(n for _, n in seg_hi)

    with ExitStack() as ctx:
        tc = ctx.enter_context(tile.TileContext(nc))
        keep = ctx.enter_context(tc.tile_pool(name="keep", bufs=1))

        # ---------------- persistent tiles (gather-critical inputs first)
        idx_sb = keep.tile([128, totch * 8], I16)
        nc.sync.dma_start(out=idx_sb[:], in_=t_idx.ap())
        dstrel_sb = keep.tile([128, totch], F32)
        nc.sync.dma_start(out=dstrel_sb[:], in_=t_dstrel.ap())
        coef_sb = keep.tile([128, totch], F32)
        nc.sync.dma_start(out=coef_sb[:], in_=t_coef.ap())
        iota_t = keep.tile([128, 128], BF)
        nc.sync.dma_start(out=iota_t[:], in_=t_iota.ap())
        wbf = keep.tile([128, 17, D], BF)
        nc.sync.dma_start(out=wbf[:], in_=t_wpack.ap().rearrange("b k m -> k b m"))
        vp = keep.tile([128, 13], F32)
        nc.sync.dma_start(out=vp[:], in_=t_vpack.ap())
        id128 = keep.tile([128, 128], BF)
        nc.sync.dma_start(out=id128[:], in_=t_id128.ap())
        id128f = keep.tile([128, 128], F32)
        nc.sync.dma_start(out=id128f[:], in_=t_id128f.ap())
        idrep_t = keep.tile([128, 16, 16], BF)
        nc.sync.dma_start(out=idrep_t[:], in_=t_idrep.ap())
        bqone_t = keep.tile([64, 12, 128], BF)
        nc.sync.dma_start(out=bqone_t[:], in_=t_bqone.ap())
        bqneg_t = keep.tile([64, 12, 128], BF)
        nc.sync.dma_start(out=bqneg_t[:], in_=t_bqneg.ap())
        xTbf = keep.tile([128, PCP], BF)
        nc.sync.dma_start(out=xTbf[:], in_=t_xT.ap())

        y0_all = keep.tile([128, PCP], BF)
        y1_all = keep.tile([128, PCP], BF)
        # Newton constants
        c_magic = keep.tile([16, 512], I32)
        nc.vector.memset(c_magic[:], RSQRT_C)
        c_one = keep.tile([16, 512], I32)
        nc.vector.memset(c_one[:], 1)

        def vcol(j):
            return vp[:, j:j + 1]

        with ExitStack() as pp:
            sb1 = pp.enter_context(tc.tile_pool(name="sb1", bufs=2))
            sb2 = pp.enter_context(tc.tile_pool(name="sb2", bufs=2))
            nwt = pp.enter_context(tc.tile_pool(name="nwt", bufs=1))
            qpl = pp.enter_context(tc.tile_pool(name="qpl", bufs=2))
            xel = pp.enter_context(tc.tile_pool(name="xel", bufs=4))
            xeh = pp.enter_context(tc.tile_pool(name="xeh", bufs=4))
            zpsp = pp.enter_context(tc.tile_pool(name="zpsp", bufs=2, space="PSUM"))
            stp = pp.enter_context(tc.tile_pool(name="stp", bufs=1, space="PSUM"))
            lnp = pp.enter_context(tc.tile_pool(name="lnp", bufs=2, space="PSUM"))
            bcp = pp.enter_context(tc.tile_pool(name="bcp", bufs=2, space="PSUM"))
            finp = pp.enter_context(tc.tile_pool(name="finp", bufs=1, space="PSUM"))

            def pass1_chunk(ci, st, cl, blen):
                g0, g1 = ci * BSEG, min((ci + 1) * BSEG, G)
                W = (g1 - g0) * 128        # scatter width (may be 128)
                n0 = g0 * 128
                last = cl == blen - 1
                segs = sorted({g // GSEG for g in range(g0, g1)})
                xe_map = {}
                for s in segs:
                    lo_start, lo_n = seg_lo[s]
                    hi_start, hi_n = seg_hi[s]
                    xe_lo = xel.tile([128, max_lo, D], BF, tag="xel")
                    xe_hi = xeh.tile([128, max_hi, D], BF, tag="xeh")
                    if lo_n:
                        nc.gpsimd.dma_gather(
                            out_ap=xe_lo[:, :lo_n, :], in_ap=t_xbf.ap()[0:HALF, :],
                            idxs_ap=idx_sb[:, lo_start * 8:(lo_start + lo_n) * 8],
                            num_idxs=lo_n * 128, num_idxs_reg=lo_n * 128,
                            elem_size=D, single_packet=False)
                    if hi_n:
                        nc.gpsimd.dma_gather(
                            out_ap=xe_hi[:, :hi_n, :], in_ap=t_xbf.ap()[HALF:NP, :],
                            idxs_ap=idx_sb[:, hi_start * 8:(hi_start + hi_n) * 8],
                            num_idxs=hi_n * 128, num_idxs_reg=hi_n * 128,
                            elem_size=D, single_packet=False)
                    xe_map[s] = (xe_lo, xe_hi)

                zps = zpsp.tile([128, 512], F32, space="PSUM", tag="zps")
                if W < 512:
                    nc.vector.memset(zps[:, W:], 0.0)
                for g in range(g0, g1):
                    col = (g - g0) * 128
                    xe_lo, xe_hi = xe_map[g // GSEG]
                    mms = []
                    for h, (xe, stt) in enumerate(
                            [(xe_lo, seg_lo[g // GSEG][0]),
                             (xe_hi, seg_hi[g // GSEG][0])]):
                        for k in range(int(cmax[g, h])):
                            gch = int(ch_off[h, g]) + k
                            sch = gch - stt
                            s_t = sb1.tile([128, 128], BF, tag="s_t")
                            # S[e, d] = (dstrel[e] == d) * norm[e]
                            nc.vector.tensor_scalar(
                                out=s_t[:], in0=iota_t[:],
                                scalar1=dstrel_sb[:, gch:gch + 1],
                                scalar2=coef_sb[:, gch:gch + 1],
                                op0=OP.is_equal, op1=OP.mult)
                            mms.append((xe, sch, s_t))
                    for mi, (xe, sch, s_t) in enumerate(mms):
                        nc.tensor.matmul(
                            out=zps[:, col:col + 128], lhsT=xe[:, sch, :],
                            rhs=s_t[:], start=(mi == 0), stop=(mi == len(mms) - 1))

                u_t = sb1.tile([128, 512], BF, tag="u")
                nc.scalar.copy(out=u_t[:], in_=zps[:])

                # agg = gcn_W^T z (+b);  topo = topo_W^T x (+b); both bf16
                aps = zpsp.tile([128, 512], F32, space="PSUM", tag="zps")
                nc.tensor.matmul(out=aps[:], lhsT=wbf[:, 0, :],
                                 rhs=u_t[:], start=True, stop=True)
                nc.scalar.activation(out=y0_all[:, n0:n0 + 512], in_=aps[:],
                                     func=AF.Identity, bias=vcol(0))
                sq = sb1.tile([128, 512], BF, tag="sq")
                nc.scalar.activation(out=sq[:], in_=aps[:],
                                     func=AF.Square, bias=vcol(0))
                nc.tensor.matmul(out=st[0:16, :], lhsT=idrep_t[:, cl, :],
                                 rhs=y0_all[:, n0:n0 + 512], start=(cl == 0),
                                 stop=False, skip_group_check=True)
                nc.tensor.matmul(out=st[32:48, :], lhsT=idrep_t[:, cl, :],
                                 rhs=sq[:], start=(cl == 0), stop=False,
                                 skip_group_check=True)

                tps = zpsp.tile([128, 512], F32, space="PSUM", tag="zps")
                nc.tensor.matmul(out=tps[:], lhsT=wbf[:, 1, :],
                                 rhs=xTbf[:, n0:n0 + 512], start=True, stop=True)
                nc.scalar.activation(out=y1_all[:, n0:n0 + 512], in_=tps[:],
                                     func=AF.Identity, bias=vcol(5))
                sqt = sb1.tile([128, 512], BF, tag="sqt")
                nc.scalar.activation(out=sqt[:], in_=tps[:],
                                     func=AF.Square, bias=vcol(5))
                nc.tensor.matmul(out=st[0:16, :], lhsT=idrep_t[:, 8 + cl, :],
                                 rhs=y1_all[:, n0:n0 + 512], start=False,
                                 stop=last, skip_group_check=True)
                nc.tensor.matmul(out=st[32:48, :], lhsT=idrep_t[:, 8 + cl, :],
                                 rhs=sqt[:], start=False, stop=last,
                                 skip_group_check=True)

            def interlude(st):
                """Q[0:16] = s1 (bf16), Q[32:48] = R' = rsqrt(128*s2 - s1^2
                + 16384*eps) = rstd/128, via DVE Newton."""
                s1c_t = nwt.tile([16, 512], F32, tag="s1c")
                q_t = nwt.tile([16, 512], F32, tag="qq")
                v_t = nwt.tile([16, 512], F32, tag="vv")
                yi_t = nwt.tile([16, 512], I32, tag="yy")
                t_t = nwt.tile([16, 512], F32, tag="tt0")
                tt_t = nwt.tile([16, 512], F32, tag="tt1")
                s1c, q, v, t, tt = s1c_t[:], q_t[:], v_t[:], t_t[:], tt_t[:]
                yi = yi_t[:]
                nc.vector.tensor_copy(out=s1c, in_=st[0:16, :])
                nc.vector.tensor_scalar(out=t, in0=st[32:48, :],
                                        scalar1=128.0, scalar2=16384.0 * EPS,
                                        op0=OP.mult, op1=OP.add)
                nc.vector.tensor_tensor(out=q, in0=s1c, in1=s1c, op=OP.mult)
                nc.vector.tensor_tensor(out=v, in0=t, in1=q, op=OP.subtract)
                # Newton rsqrt: y0 = bits(C - (bits(v) >> 1))
                nc.vector.tensor_tensor(out=yi, in0=v.bitcast(I32),
                                        in1=c_one[:], op=OP.arith_shift_right)
                nc.vector.tensor_tensor(out=yi, in0=c_magic[:], in1=yi,
                                        op=OP.subtract)
                y = yi.bitcast(F32)
                for _ in range(NEWTON_ITERS):
                    nc.vector.tensor_tensor(out=tt, in0=y, in1=y, op=OP.mult)
                    nc.vector.tensor_tensor(out=tt, in0=tt, in1=v, op=OP.mult)
                    nc.vector.tensor_scalar(out=tt, in0=tt, scalar1=-0.5,
                                            scalar2=1.5, op0=OP.mult, op1=OP.add)
                    nc.vector.tensor_tensor(out=y, in0=y, in1=tt, op=OP.mult)
                qt = qpl.tile([64, 512], BF, tag="q")
                nc.vector.memset(qt[:], 0.0)
                nc.scalar.copy(out=qt[0:16, :], in_=s1c)
                nc.scalar.copy(out=qt[32:48, :], in_=yi.bitcast(F32))
                return qt

            def pass2_chunk(ci, cl, qt):
                g0, g1 = ci * BSEG, min((ci + 1) * BSEG, G)
                W = (g1 - g0) * 128
                n0 = g0 * 128

                def layernorm(yall, k, g_j, b_j, ln):
                    yps = lnp.tile([128, 512], F32, space="PSUM", tag="ln")
                    nc.tensor.matmul(out=yps[:], lhsT=id128[:],
                                     rhs=yall[:, n0:n0 + 512], start=True,
                                     stop=False)
                    nc.tensor.matmul(out=yps[:], lhsT=bqneg_t[:, k, :],
                                     rhs=qt[:], start=False, stop=True)
                    rb = bcp.tile([128, 512], F32, space="PSUM", tag="bc")
                    nc.tensor.matmul(out=rb[:], lhsT=bqone_t[:, k, :],
                                     rhs=qt[:], start=True, stop=True)
                    rbg = sb2.tile([128, 512], BF, tag="rbg" + str(ln))
                    nc.scalar.activation(out=rbg[:], in_=rb[:],
                                         func=AF.Identity, scale=vcol(g_j))
                    t1 = sb2.tile([128, 512], BF, tag="t1" + str(ln))
                    nc.vector.tensor_tensor(out=t1[:], in0=yps[:], in1=rbg[:],
                                            op=OP.mult)
                    o = sb2.tile([128, 512], BF, tag="nf" + str(ln))
                    nc.scalar.activation(out=o[:], in_=t1[:], func=AF.Relu,
                                         bias=vcol(b_j))
                    return o

                nf = layernorm(y0_all, cl, 1, 2, 0)
                tf = layernorm(y1_all, 8 + cl, 3, 4, 1)

                diff = sb2.tile([128, 512], BF, tag="diff")
                nc.vector.tensor_tensor(out=diff[:], in0=nf[:], in1=tf[:],
                                        op=OP.subtract)

                h1ps = finp.tile([128, 512], F32, space="PSUM", tag="fin")
                nc.tensor.matmul(out=h1ps[:], lhsT=wbf[:, 16, :], rhs=tf[:],
                                 start=True, stop=False)
                for h in range(H):
                    gps = bcp.tile([128, 512], F32, space="PSUM", tag="bc")
                    nc.tensor.matmul(out=gps[:], lhsT=wbf[:, 8 + 2 * h, :],
                                     rhs=nf[:], start=True, stop=False)
                    nc.tensor.matmul(out=gps[:], lhsT=wbf[:, 9 + 2 * h, :],
                                     rhs=tf[:], start=False, stop=True)
                    gate = sb2.tile([128, 512], BF, tag="gate")
                    nc.scalar.activation(out=gate[:], in_=gps[:],
                                         func=AF.Sigmoid, bias=vcol(6 + h))
                    fh = sb2.tile([128, 512], BF, tag="fh")
                    nc.vector.tensor_tensor(out=fh[:], in0=gate[:],
                                            in1=diff[:], op=OP.mult)
                    nc.tensor.matmul(out=h1ps[:], lhsT=wbf[:, 4 + h, :],
                                     rhs=fh[:], start=False, stop=(h == H - 1))

                h1 = sb2.tile([128, 512], BF, tag="h1")
                nc.scalar.activation(out=h1[:], in_=h1ps[:],
                                     func=AF.Relu, bias=vcol(10))
                h2ps = finp.tile([128, 512], F32, space="PSUM", tag="fin")
                nc.tensor.matmul(out=h2ps[:], lhsT=wbf[:, 3, :],
                                 rhs=h1[:], start=True, stop=True)
                rps = lnp.tile([128, 512], F32, space="PSUM", tag="ln")
                nc.tensor.matmul(out=rps[:], lhsT=wbf[:, 2, :],
                                 rhs=xTbf[:, n0:n0 + 512], start=True, stop=True)
                rph = sb2.tile([128, 512], BF, tag="rph")
                nc.scalar.activation(out=rph[:], in_=rps[:],
                                     func=AF.Identity, bias=vcol(12))
                h2 = sb2.tile([128, 512], BF, tag="h2")
                nc.vector.tensor_scalar(out=h2[:], in0=h2ps[:],
                                        scalar1=vcol(11), scalar2=0.0,
                                        op0=OP.add, op1=OP.max)
                o = sb2.tile([128, 512], F32, tag="o")
                nc.gpsimd.tensor_tensor(out=o[:], in0=h2[:], in1=rph[:],
                                        op=OP.add)

                ng = g1 - g0
                otp = finp.tile([128, 4, 128], F32, space="PSUM", tag="fin")
                for g in range(g0, g1):
                    col = (g - g0) * 128
                    nc.tensor.transpose(out=otp[:, g - g0, :],
                                        in_=o[:, col:col + 128],
                                        identity=id128f[:])
                orow = sb2.tile([128, 4, 128], F32, tag="orow")
                nc.scalar.copy(out=orow[:, :ng, :], in_=otp[:, :ng, :])
                nc.sync.dma_start(
                    out=t_out.ap()[n0:n0 + W, :].rearrange(
                        "(g p) d -> p g d", p=128),
                    in_=orow[:, :ng, :])

            starts = np.concatenate([[0], np.cumsum(BATCHES)]).astype(int)
            nb = len(BATCHES)
            qts = [None] * nb
            stv = [None] * nb
            for _b in range(nb):
                stt = stp.tile([48, 512], F32, space="PSUM", tag="st")
                stv[_b] = stt

            def stview(b):
                return stv[b]

            def batch_of(ci):
                return int(np.searchsorted(starts, ci, side="right") - 1)

            # greedy schedule: pass1 priority, interludes ASAP, pass2 fills in
            for ci in range(BATCHES[0]):
                pass1_chunk(ci, stview(0), ci, BATCHES[0])
            qts[0] = interlude(stview(0))
            p1n = starts[1]          # next pass1 chunk to emit
            p2n = 0                  # next pass2 chunk to emit
            while p1n < NCHUNK or p2n < NCHUNK:
                if p1n < NCHUNK:
                    b = batch_of(p1n)
                    pass1_chunk(p1n, stview(b), p1n - starts[b], BATCHES[b])
                    if p1n == starts[b + 1] - 1:
                        qts[b] = interlude(stview(b))
                    p1n += 1
                nemit = 1 + (p1n - p2n >= 5)
                for _ in range(nemit):
                    if p2n < NCHUNK:
                        b2 = batch_of(p2n)
                        if qts[b2] is not None:
                            pass2_chunk(p2n, p2n - starts[b2], qts[b2])
                            p2n += 1

    nc.compile()
    return nc


# ---------------------------------------------------------------- entry
LAST_RESULTS = None
LAST_NC = None
LAST_INMAPS = None


def kernel(**inputs):
    import os
    from concourse.bass_utils import run_bass_kernel_spmd

    x = np.asarray(inputs["x"], dtype=np.float32)
    x_pad, x_bf, idx16_all, dstrel_all, coef_all, meta = _prep(
        x, inputs["edge_index"])
    wpack, vpack = _pack_weights({k: np.asarray(v, dtype=np.float32)
                                  for k, v in inputs.items() if k != "edge_index"})

    nc = _build(meta)

    in_maps = []
    for c in range(NCORES):
        xT = np.zeros((D, PCP), dtype=BF16)
        xT[:, :PC] = x_pad[c * PC:(c + 1) * PC].T.astype(BF16)
        in_maps.append({
            "xbf": x_bf, "xT": xT,
            "idx16": idx16_all[c], "dstrel": dstrel_all[c], "coef": coef_all[c],
            "wpack": wpack, "vpack": vpack,
        })
    global LAST_RESULTS, LAST_NC, LAST_INMAPS
    LAST_NC, LAST_INMAPS = nc, in_maps
    res = run_bass_kernel_spmd(nc, in_maps, core_ids=list(range(NCORES)),
                               trace=bool(os.environ.get("KTRACE")))
    LAST_RESULTS = res
    out = np.concatenate([res.results[c]["out"] for c in range(NCORES)], axis=0)
    return out[:N].astype(np.float32)


# revision 43
# speedup vs baseline: 1.5022x; 1.2645x over previous
"""EnhancedGTATLayer Trainium2 kernel — 8-core SPMD Bass implementation.

Host: sorts edges by (dst-group, src-half), pads to a uniform cross-core
chunk structure (one SPMD NEFF), packs int16 gather indices and per-slot
(one-hot dst, GCN-norm coefficient) pairs.  x is shipped twice: full copy
in bf16 (gather source) and a per-core feature-transposed slice in bf16.

Device (per core, 6272 dst nodes = 49 groups of 128), feature-transposed
[feat, node] layout, bf16 matmuls throughout (PSUM accumulate fp32):
  pass 1: dma_gather x rows by src (bf16, 256B rows); S[e,d] =
          (dstrel[e]==d)*norm[e] built on DVE in bf16 (4x mode);
          z^T += Xe^T S in PSUM; y0 = agg = gcn_W^T z + b and
          y1 = topo_W^T x + b persisted in bf16; LN stats (sum, sumsq)
          via one-hot selector matmuls into a [32,512] PSUM tile.
  interlude (per 7-chunk batch): R' = rsqrt(128*s2 - s1^2 + 16384*eps)
          (= rstd/128) via DVE Newton; Q[0:16]=s1, Q[16:32]=R' in bf16.
  pass 2: yps = I@y - mu (broadcast matmul accumulated in PSUM);
          ln = Act(yps * R'_bcast, Relu, scale=128*g, bias=beta);
          sigmoid gates; fused = Sum_h W1_h^T(gate_h*diff) + W1s^T topo;
          MLP, residual; PE-transpose to row layout, one DMA per chunk.
"""
import sys

sys.path.insert(0, "/opt/trn_rl_repo")

import os
import numpy as np
import ml_dtypes

BF16 = ml_dtypes.bfloat16
GMODE = os.environ.get("GMODE", "full")  # full | half | off (timing experiments)

N = 50000
NP = 50176          # padded to 392*128
PC = 6272           # nodes per core = 49*128
PCP = 6656          # padded to 13*512 for uniform 512-wide chunks
NCORES = 8
G = 49              # dst groups of 128 per core
D = 128             # feature dim (CIN == COUT)
H = 4
EPS = 1e-5
HALF = 32768        # int16 index split
GSEG = 1            # dst groups per gather segment
NSEG = 49           # one per group
BSEG = 4            # dst groups per chunk (512 cols)
NCHUNK = 13         # ceil(49/4); last chunk has 1 group
BATCHES = [4, 4, 3, 2]  # pipelined chunk batches
RSQRT_C = 0x5F3759DF
NEWTON_ITERS = 1


# ---------------------------------------------------------------- host prep
def _prep(x, edge_index):
    src = np.asarray(edge_index[0], dtype=np.int64)
    dst = np.asarray(edge_index[1], dtype=np.int64)
    loops = np.arange(NP, dtype=np.int64)
    src_all = np.concatenate([src, loops])
    dst_all = np.concatenate([dst, loops])

    deg = np.bincount(dst_all, minlength=NP)
    x_pad = np.zeros((NP, D), dtype=np.float32)
    x_pad[:N] = np.asarray(x, dtype=np.float32)

    core_of = dst_all // PC
    per_core = []
    counts = np.zeros((NCORES, G, 2), dtype=np.int64)
    for c in range(NCORES):
        m = core_of == c
        s = src_all[m]
        dl = dst_all[m] - c * PC
        g = dl >> 7
        h = (s >= HALF).astype(np.int64)
        order = np.lexsort((s, h, g))
        s, dl, h = s[order], dl[order], h[order]
        key = (dl >> 7) * 2 + h
        counts[c] = np.bincount(key, minlength=G * 2).reshape(G, 2)
        per_core.append((s, dl, key))

    cmax = ((counts + 127) // 128).max(axis=0)               # [G, 2] chunks
    cntmax = counts.max(axis=0)                              # [G, 2] rows
    ch_off = np.zeros((2, G), dtype=np.int64)
    ch_off[0] = np.concatenate([[0], np.cumsum(cmax[:, 0])[:-1]])
    nch_lo = int(cmax[:, 0].sum())
    ch_off[1] = nch_lo + np.concatenate([[0], np.cumsum(cmax[:, 1])[:-1]])
    totch = nch_lo + int(cmax[:, 1].sum())
    totslots = totch * 128

    dis = deg.astype(np.float32) ** -0.5   # reference: deg ** -0.5 in f32

    idx16_all, dstrel_all, coef_all = [], [], []
    for c in range(NCORES):
        s, dl, key = per_core[c]
        idx = np.full(totslots, -1, dtype=np.int16)
        dr = np.full(totslots, -1.0, dtype=np.float32)
        cf = np.zeros(totslots, dtype=np.float32)
        starts = np.concatenate([[0], np.cumsum(np.bincount(key, minlength=G * 2))])
        for g in range(G):
            for h in range(2):
                a, b = starts[g * 2 + h], starts[g * 2 + h + 1]
                off = ch_off[h, g] * 128
                # real edges, then dummy-valid rows up to the cross-core max
                # count (uniform num_idxs_reg), then -1 tail (not transferred)
                idx[off:off + (b - a)] = (s[a:b] - (HALF if h else 0)).astype(np.int16)
                idx[off + (b - a):off + int(cntmax[g, h])] = 0
                if b > a:
                    dr[off:off + (b - a)] = (dl[a:b] & 127).astype(np.float32)
                    cf[off:off + (b - a)] = dis[s[a:b]] * dis[dl[a:b] + c * PC]
        idx16_all.append(np.tile(idx.reshape(-1, 16).T, (8, 1)).astype(np.int16))
        dstrel_all.append(np.ascontiguousarray(dr.reshape(totch, 128).T))
        coef_all.append(np.ascontiguousarray(cf.reshape(totch, 128).T))

    meta = dict(cmax=cmax, ch_off=ch_off, totch=totch, cntmax=cntmax)
    x_bf = x_pad.astype(BF16)
    return x_pad, x_bf, idx16_all, dstrel_all, coef_all, meta


def _pack_weights(ins):
    w = np.zeros((17, D, D), dtype=np.float32)
    w[0] = ins["gcn_W"]
    w[1] = ins["topo_W"]
    w[2] = ins["res_W"]
    w[3] = ins["mlp_W2"]
    for h in range(H):
        w[4 + h] = ins["mlp_W1"][h * D:(h + 1) * D, :]
        w[8 + 2 * h] = ins["attn_W"][h][:D, :]
        w[9 + 2 * h] = ins["attn_W"][h][D:, :]
        w[16] += ins["mlp_W1"][h * D:(h + 1) * D, :]
    v = np.zeros((D, 13), dtype=np.float32)
    v[:, 0] = ins["gcn_b"]
    v[:, 1] = 128.0 * ins["ln_node_g"]
    v[:, 2] = ins["ln_node_b"]
    v[:, 3] = 128.0 * ins["ln_topo_g"]
    v[:, 4] = ins["ln_topo_b"]
    v[:, 5] = ins["topo_b"]
    for h in range(H):
        v[:, 6 + h] = ins["attn_b"][h]
    v[:, 10] = ins["mlp_b1"]
    v[:, 11] = ins["mlp_b2"]
    v[:, 12] = ins["res_b"]
    return w.astype(BF16), v


# ---------------------------------------------------------------- device
def _build(meta):
    import concourse.bacc as bacc
    import concourse.tile as tile
    from concourse import mybir
    from contextlib import ExitStack

    cmax, ch_off, totch = meta["cmax"], meta["ch_off"], meta["totch"]
    F32, BF, I16, I32 = (mybir.dt.float32, mybir.dt.bfloat16,
                         mybir.dt.int16, mybir.dt.int32)
    AF = mybir.ActivationFunctionType
    OP = mybir.AluOpType

    nc = bacc.Bacc("TRN2", target_bir_lowering=False, num_devices=NCORES,
                   dynamic_dma_scratch_size=65536, num_swdge_queues=4)
    t_xbf = nc.dram_tensor("xbf", [NP, D], BF, kind="ExternalInput")
    t_xT = nc.dram_tensor("xT", [D, PCP], BF, kind="ExternalInput")
    t_idx = nc.dram_tensor("idx16", [128, totch * 8], I16, kind="ExternalInput")
    t_dstrel = nc.dram_tensor("dstrel", [128, totch], F32, kind="ExternalInput")
    t_coef = nc.dram_tensor("coef", [128, totch], F32, kind="ExternalInput")
    t_wpack = nc.dram_tensor("wpack", [17, D, D], BF, kind="ExternalInput")
    t_vpack = nc.dram_tensor("vpack", [D, 13], F32, kind="ExternalInput")
    t_brow = nc.dram_tensor("brow", [1, D], BF, kind="ExternalInput")
    t_out = nc.dram_tensor("out", [PC, D], F32, kind="ExternalOutput")

    iota_np = np.broadcast_to(np.arange(128, dtype=np.float32),
                              (128, 128)).astype(BF16)
    t_iota = nc.inline_tensor(iota_np.copy(), name="iota128")
    t_id128 = nc.inline_tensor(np.eye(128, dtype=np.float32).astype(BF16),
                               name="ident128")
    t_id128f = nc.inline_tensor(np.eye(128, dtype=np.float32), name="ident128f")
    # stats selectors: [128, 16, 16], [:, k, j] = (j == k)
    idrep = np.broadcast_to(np.eye(16, dtype=np.float32), (128, 16, 16))
    t_idrep = nc.inline_tensor(idrep.astype(BF16).copy(), name="idrep16")
    # broadcast selectors over the [32, 512] Q tile (rows 0:16 = s1 sums,
    # rows 16:32 = R'):  bqone picks R' row, bqneg adds -s1/128 (= -mu).
    bqone = np.zeros((64, 12, 128), dtype=np.float32)
    bqneg = np.zeros((64, 12, 128), dtype=np.float32)
    for k in range(12):
        bqone[32 + k, k, :] = 1.0
        bqneg[k, k, :] = -1.0 / 128.0
    t_bqone = nc.inline_tensor(bqone.astype(BF16), name="bqone")
    t_bqneg = nc.inline_tensor(bqneg.astype(BF16), name="bqneg")

    # gather segment geometry (uniform across cores)
    cntmax = meta["cntmax"]
    seg_lo, seg_hi = [], []
    for s in range(NSEG):
        g0, g1 = s * GSEG, min((s + 1) * GSEG, G)
        seg_lo.append((int(ch_off[0, g0]), int(cmax[g0:g1, 0].sum()),
                       int(cntmax[g0:g1, 0].sum())))
        seg_hi.append((int(ch_off[1, g0]), int(cmax[g0:g1, 1].sum()),
                       int(cntmax[g0:g1, 1].sum())))
    max_lo = max(n for _, n, _ in seg_lo)
    max_hi = max(n for _, n, _ in seg_hi)

    with ExitStack() as ctx:
        tc = ctx.enter_context(tile.TileContext(nc))
        keep = ctx.enter_context(tc.tile_pool(name="keep", bufs=1))

        # ---------------- persistent tiles (gather-critical inputs first)
        idx_sb = keep.tile([128, totch * 8], I16)
        nc.sync.dma_start(out=idx_sb[:], in_=t_idx.ap())
        dstrel_sb = keep.tile([128, totch], F32)
        nc.sync.dma_start(out=dstrel_sb[:], in_=t_dstrel.ap())
        coef_sb = keep.tile([128, totch], F32)
        nc.sync.dma_start(out=coef_sb[:], in_=t_coef.ap())
        iota_t = keep.tile([128, 128], BF)
        nc.sync.dma_start(out=iota_t[:], in_=t_iota.ap())
        wbf = keep.tile([128, 17, D], BF)
        nc.sync.dma_start(out=wbf[:], in_=t_wpack.ap().rearrange("b k m -> k b m"))
        vp = keep.tile([128, 13], F32)
        nc.sync.dma_start(out=vp[:], in_=t_vpack.ap())
        id128 = keep.tile([128, 128], BF)
        nc.sync.dma_start(out=id128[:], in_=t_id128.ap())
        id128f = keep.tile([128, 128], F32)
        nc.sync.dma_start(out=id128f[:], in_=t_id128f.ap())
        idrep_t = keep.tile([128, 16, 16], BF)
        nc.sync.dma_start(out=idrep_t[:], in_=t_idrep.ap())
        bqone_t = keep.tile([64, 12, 128], BF)
        nc.sync.dma_start(out=bqone_t[:], in_=t_bqone.ap())
        bqneg_t = keep.tile([64, 12, 128], BF)
        nc.sync.dma_start(out=bqneg_t[:], in_=t_bqneg.ap())
        xTbf = keep.tile([128, PCP], BF)
        nc.sync.dma_start(out=xTbf[:], in_=t_xT.ap())

        y0_all = keep.tile([128, PCP], BF)
        brow = keep.tile([1, D], BF)
        nc.sync.dma_start(out=brow[:], in_=t_brow.ap())
        ones_row = keep.tile([1, 512], BF)
        nc.vector.memset(ones_row[:], 1.0)
        # Newton constants
        c_magic = keep.tile([16, 512], I32)
        nc.vector.memset(c_magic[:], RSQRT_C)
        c_one = keep.tile([16, 512], I32)
        nc.vector.memset(c_one[:], 1)

        def vcol(j):
            return vp[:, j:j + 1]

        with ExitStack() as pp:
            sb1 = pp.enter_context(tc.tile_pool(name="sb1", bufs=2))
            sb2 = pp.enter_context(tc.tile_pool(name="sb2", bufs=2))
            nwt = pp.enter_context(tc.tile_pool(name="nwt", bufs=1))
            qpl = pp.enter_context(tc.tile_pool(name="qpl", bufs=2))
            xel = pp.enter_context(tc.tile_pool(name="xel", bufs=10))
            xeh = pp.enter_context(tc.tile_pool(name="xeh", bufs=10))
            zpsp = pp.enter_context(tc.tile_pool(name="zpsp", bufs=2, space="PSUM"))
            stp = pp.enter_context(tc.tile_pool(name="stp", bufs=1, space="PSUM"))
            lnp = pp.enter_context(tc.tile_pool(name="lnp", bufs=2, space="PSUM"))
            bcp = pp.enter_context(tc.tile_pool(name="bcp", bufs=2, space="PSUM"))
            finp = pp.enter_context(tc.tile_pool(name="finp", bufs=1, space="PSUM"))

            gq = [0]  # emitted-gather counter: queue = gq % 4 keeps the
                      # round-robin DMA-sem assignment queue-consistent
            warmed = []
            for _w in range(10):
                xwl = xel.tile([128, max_lo, D], BF, tag="xel")
                nc.vector.memset(xwl[:], 0.0)
                xwh = xeh.tile([128, max_hi, D], BF, tag="xeh")
                nc.vector.memset(xwh[:], 0.0)
                warmed.append((xwl, xwh))

            def pass1_chunk(ci, st, cl, blen):
                g0, g1 = ci * BSEG, min((ci + 1) * BSEG, G)
                W = (g1 - g0) * 128        # scatter width (may be 128)
                n0 = g0 * 128
                last = cl == blen - 1
                segs = sorted({g // GSEG for g in range(g0, g1)})
                xe_map = {}
                for s in segs:
                    lo_start, lo_n, lo_cnt = seg_lo[s]
                    hi_start, hi_n, hi_cnt = seg_hi[s]
                    xe_lo = xel.tile([128, max_lo, D], BF, tag="xel")
                    xe_hi = xeh.tile([128, max_hi, D], BF, tag="xeh")
                    esz = D
                    if GMODE == "off":
                        nc.vector.memset(xe_lo[:], 0.0)
                        nc.vector.memset(xe_hi[:], 0.0)
                    if lo_n and lo_cnt and GMODE != "off":
                        nc.gpsimd.dma_gather(
                            out_ap=xe_lo[:, :lo_n, :esz],
                            in_ap=t_xbf.ap()[0:HALF, 0:esz],
                            idxs_ap=idx_sb[:, lo_start * 8:(lo_start + lo_n) * 8],
                            num_idxs=lo_n * 128, num_idxs_reg=lo_cnt,
                            elem_size=esz, elem_step=D, single_packet=False,
                            queue_num=gq[0] % 4)
                        gq[0] += 1
                    if hi_n and hi_cnt and GMODE != "off":
                        nc.gpsimd.dma_gather(
                            out_ap=xe_hi[:, :hi_n, :esz],
                            in_ap=t_xbf.ap()[HALF:NP, 0:esz],
                            idxs_ap=idx_sb[:, hi_start * 8:(hi_start + hi_n) * 8],
                            num_idxs=hi_n * 128, num_idxs_reg=hi_cnt,
                            elem_size=esz, elem_step=D, single_packet=False,
                            queue_num=gq[0] % 4)
                        gq[0] += 1
                    xe_map[s] = (xe_lo, xe_hi)

                zps = zpsp.tile([128, 512], F32, space="PSUM", tag="zps")
                if W < 512:
                    nc.vector.memset(zps[:, W:], 0.0)
                for g in range(g0, g1):
                    col = (g - g0) * 128
                    xe_lo, xe_hi = xe_map[g // GSEG]
                    mms = []
                    for h, (xe, stt) in enumerate(
                            [(xe_lo, seg_lo[g // GSEG][0]),
                             (xe_hi, seg_hi[g // GSEG][0])]):
                        for k in range(int(cmax[g, h])):
                            gch = int(ch_off[h, g]) + k
                            sch = gch - stt
                            s_t = sb1.tile([128, 128], BF, tag="s_t")
                            # S[e, d] = (dstrel[e] == d) * norm[e]
                            nc.vector.tensor_scalar(
                                out=s_t[:], in0=iota_t[:],
                                scalar1=dstrel_sb[:, gch:gch + 1],
                                scalar2=coef_sb[:, gch:gch + 1],
                                op0=OP.is_equal, op1=OP.mult)
                            mms.append((xe, sch, s_t))
                    for mi, (xe, sch, s_t) in enumerate(mms):
                        nc.tensor.matmul(
                            out=zps[:, col:col + 128], lhsT=xe[:, sch, :],
                            rhs=s_t[:], start=(mi == 0), stop=(mi == len(mms) - 1))

                u_t = sb1.tile([128, 512], BF, tag="u")
                nc.scalar.copy(out=u_t[:], in_=zps[:])

                # agg = gcn_W^T z (+b);  topo = topo_W^T x (+b); both bf16
                aps = zpsp.tile([128, 512], F32, space="PSUM", tag="zps")
                nc.tensor.matmul(out=aps[:], lhsT=wbf[:, 0, :],
                                 rhs=u_t[:], start=True, stop=True)
                nc.scalar.activation(out=y0_all[:, n0:n0 + 512], in_=aps[:],
                                     func=AF.Identity, bias=vcol(0))
                sq = sb1.tile([128, 512], BF, tag="sq")
                nc.scalar.activation(out=sq[:], in_=aps[:],
                                     func=AF.Square, bias=vcol(0))
                nc.tensor.matmul(out=st[0:16, :], lhsT=idrep_t[:, cl, :],
                                 rhs=y0_all[:, n0:n0 + 512], start=(cl == 0),
                                 stop=False, skip_group_check=True)
                nc.tensor.matmul(out=st[32:48, :], lhsT=idrep_t[:, cl, :],
                                 rhs=sq[:], start=(cl == 0), stop=False,
                                 skip_group_check=True)

                tps = zpsp.tile([128, 512], F32, space="PSUM", tag="zps")
                nc.tensor.matmul(out=tps[:], lhsT=wbf[:, 1, :],
                                 rhs=xTbf[:, n0:n0 + 512], start=True, stop=True)
                y1 = sb1.tile([128, 512], BF, tag="y1")
                nc.scalar.activation(out=y1[:], in_=tps[:],
                                     func=AF.Identity, bias=vcol(5))
                sqt = sb1.tile([128, 512], BF, tag="sqt")
                nc.scalar.activation(out=sqt[:], in_=tps[:],
                                     func=AF.Square, bias=vcol(5))
                nc.tensor.matmul(out=st[0:16, :], lhsT=idrep_t[:, 8 + cl, :],
                                 rhs=y1[:], start=False,
                                 stop=last, skip_group_check=True)
                nc.tensor.matmul(out=st[32:48, :], lhsT=idrep_t[:, 8 + cl, :],
                                 rhs=sqt[:], start=False, stop=last,
                                 skip_group_check=True)

            def interlude(st):
                """Q[0:16] = s1 (bf16), Q[32:48] = R' = rsqrt(128*s2 - s1^2
                + 16384*eps) = rstd/128, via DVE Newton."""
                s1c_t = nwt.tile([16, 512], F32, tag="s1c")
                q_t = nwt.tile([16, 512], F32, tag="qq")
                v_t = nwt.tile([16, 512], F32, tag="vv")
                yi_t = nwt.tile([16, 512], I32, tag="yy")
                t_t = nwt.tile([16, 512], F32, tag="tt0")
                tt_t = nwt.tile([16, 512], F32, tag="tt1")
                s1c, q, v, t, tt = s1c_t[:], q_t[:], v_t[:], t_t[:], tt_t[:]
                yi = yi_t[:]
                nc.vector.tensor_copy(out=s1c, in_=st[0:16, :])
                nc.vector.tensor_scalar(out=t, in0=st[32:48, :],
                                        scalar1=128.0, scalar2=16384.0 * EPS,
                                        op0=OP.mult, op1=OP.add)
                nc.vector.tensor_tensor(out=q, in0=s1c, in1=s1c, op=OP.mult)
                nc.vector.tensor_tensor(out=v, in0=t, in1=q, op=OP.subtract)
                # Newton rsqrt: y0 = bits(C - (bits(v) >> 1))
                nc.vector.tensor_tensor(out=yi, in0=v.bitcast(I32),
                                        in1=c_one[:], op=OP.arith_shift_right)
                nc.vector.tensor_tensor(out=yi, in0=c_magic[:], in1=yi,
                                        op=OP.subtract)
                y = yi.bitcast(F32)
                for _ in range(NEWTON_ITERS):
                    nc.vector.tensor_tensor(out=tt, in0=y, in1=y, op=OP.mult)
                    nc.vector.tensor_tensor(out=tt, in0=tt, in1=v, op=OP.mult)
                    nc.vector.tensor_scalar(out=tt, in0=tt, scalar1=-0.5,
                                            scalar2=1.5, op0=OP.mult, op1=OP.add)
                    nc.vector.tensor_tensor(out=y, in0=y, in1=tt, op=OP.mult)
                qt = qpl.tile([64, 512], BF, tag="q")
                nc.vector.memset(qt[:], 0.0)
                nc.scalar.copy(out=qt[0:16, :], in_=s1c)
                nc.scalar.copy(out=qt[32:48, :], in_=yi.bitcast(F32))
                return qt

            def pass2_chunk(ci, cl, qt):
                g0, g1 = ci * BSEG, min((ci + 1) * BSEG, G)
                W = (g1 - g0) * 128
                n0 = g0 * 128

                def layernorm(yall, k, g_j, b_j, ln):
                    yps = lnp.tile([128, 512], F32, space="PSUM", tag="ln")
                    if yall is None:
                        # recompute topo = topo_W^T x + b (rank-1 bias bcast)
                        nc.tensor.matmul(out=yps[:], lhsT=wbf[:, 1, :],
                                         rhs=xTbf[:, n0:n0 + 512], start=True,
                                         stop=False)
                        nc.tensor.matmul(out=yps[:], lhsT=brow[:],
                                         rhs=ones_row[:], start=False,
                                         stop=False, skip_group_check=True)
                    else:
                        nc.tensor.matmul(out=yps[:], lhsT=id128[:],
                                         rhs=yall[:, n0:n0 + 512], start=True,
                                         stop=False)
                    nc.tensor.matmul(out=yps[:], lhsT=bqneg_t[:, k, :],
                                     rhs=qt[:], start=False, stop=True)
                    rb = bcp.tile([128, 512], F32, space="PSUM", tag="bc")
                    nc.tensor.matmul(out=rb[:], lhsT=bqone_t[:, k, :],
                                     rhs=qt[:], start=True, stop=True)
                    rbg = sb2.tile([128, 512], BF, tag="rbg" + str(ln))
                    nc.scalar.activation(out=rbg[:], in_=rb[:],
                                         func=AF.Identity, scale=vcol(g_j))
                    t1 = sb2.tile([128, 512], BF, tag="t1" + str(ln))
                    nc.vector.tensor_tensor(out=t1[:], in0=yps[:], in1=rbg[:],
                                            op=OP.mult)
                    o = sb2.tile([128, 512], BF, tag="nf" + str(ln))
                    nc.scalar.activation(out=o[:], in_=t1[:], func=AF.Relu,
                                         bias=vcol(b_j))
                    return o

                nf = layernorm(y0_all, cl, 1, 2, 0)
                tf = layernorm(None, 8 + cl, 3, 4, 1)

                diff = sb2.tile([128, 512], BF, tag="diff")
                nc.vector.tensor_tensor(out=diff[:], in0=nf[:], in1=tf[:],
                                        op=OP.subtract)

                h1ps = finp.tile([128, 512], F32, space="PSUM", tag="fin")
                nc.tensor.matmul(out=h1ps[:], lhsT=wbf[:, 16, :], rhs=tf[:],
                                 start=True, stop=False)
                for h in range(H):
                    gps = bcp.tile([128, 512], F32, space="PSUM", tag="bc")
                    nc.tensor.matmul(out=gps[:], lhsT=wbf[:, 8 + 2 * h, :],
                                     rhs=nf[:], start=True, stop=False)
                    nc.tensor.matmul(out=gps[:], lhsT=wbf[:, 9 + 2 * h, :],
                                     rhs=tf[:], start=False, stop=True)
                    gate = sb2.tile([128, 512], BF, tag="gate")
                    nc.scalar.activation(out=gate[:], in_=gps[:],
                                         func=AF.Sigmoid, bias=vcol(6 + h))
                    fh = sb2.tile([128, 512], BF, tag="fh")
                    nc.vector.tensor_tensor(out=fh[:], in0=gate[:],
                                            in1=diff[:], op=OP.mult)
                    nc.tensor.matmul(out=h1ps[:], lhsT=wbf[:, 4 + h, :],
                                     rhs=fh[:], start=False, stop=(h == H - 1))

                h1 = sb2.tile([128, 512], BF, tag="h1")
                nc.scalar.activation(out=h1[:], in_=h1ps[:],
                                     func=AF.Relu, bias=vcol(10))
                h2ps = finp.tile([128, 512], F32, space="PSUM", tag="fin")
                nc.tensor.matmul(out=h2ps[:], lhsT=wbf[:, 3, :],
                                 rhs=h1[:], start=True, stop=True)
                rps = lnp.tile([128, 512], F32, space="PSUM", tag="ln")
                nc.tensor.matmul(out=rps[:], lhsT=wbf[:, 2, :],
                                 rhs=xTbf[:, n0:n0 + 512], start=True, stop=True)
                rph = sb2.tile([128, 512], BF, tag="rph")
                nc.scalar.activation(out=rph[:], in_=rps[:],
                                     func=AF.Identity, bias=vcol(12))
                h2 = sb2.tile([128, 512], BF, tag="h2")
                nc.vector.tensor_scalar(out=h2[:], in0=h2ps[:],
                                        scalar1=vcol(11), scalar2=0.0,
                                        op0=OP.add, op1=OP.max)
                o = sb2.tile([128, 512], F32, tag="o")
                nc.gpsimd.tensor_tensor(out=o[:], in0=h2[:], in1=rph[:],
                                        op=OP.add)

                ng = g1 - g0
                otp = finp.tile([128, 4, 128], F32, space="PSUM", tag="fin")
                for g in range(g0, g1):
                    col = (g - g0) * 128
                    nc.tensor.transpose(out=otp[:, g - g0, :],
                                        in_=o[:, col:col + 128],
                                        identity=id128f[:])
                orow = sb2.tile([128, 4, 128], F32, tag="orow")
                nc.scalar.copy(out=orow[:, :ng, :], in_=otp[:, :ng, :])
                nc.sync.dma_start(
                    out=t_out.ap()[n0:n0 + W, :].rearrange(
                        "(g p) d -> p g d", p=128),
                    in_=orow[:, :ng, :])

            starts = np.concatenate([[0], np.cumsum(BATCHES)]).astype(int)
            nb = len(BATCHES)
            qts = [None] * nb
            stv = [None] * nb
            for _b in range(nb):
                stt = stp.tile([48, 512], F32, space="PSUM", tag="st")
                stv[_b] = stt

            def stview(b):
                return stv[b]

            def batch_of(ci):
                return int(np.searchsorted(starts, ci, side="right") - 1)

            # greedy schedule: pass1 priority, interludes ASAP, pass2 fills in
            for ci in range(BATCHES[0]):
                pass1_chunk(ci, stview(0), ci, BATCHES[0])
            qts[0] = interlude(stview(0))
            p1n = starts[1]          # next pass1 chunk to emit
            p2n = 0                  # next pass2 chunk to emit
            while p1n < NCHUNK or p2n < NCHUNK:
                if p1n < NCHUNK:
                    b = batch_of(p1n)
                    pass1_chunk(p1n, stview(b), p1n - starts[b], BATCHES[b])
                    if p1n == starts[b + 1] - 1:
                        qts[b] = interlude(stview(b))
                    p1n += 1
                nemit = 1 + (p1n - p2n >= 5)
                for _ in range(nemit):
                    if p2n < NCHUNK:
                        b2 = batch_of(p2n)
                        if qts[b2] is not None:
                            pass2_chunk(p2n, p2n - starts[b2], qts[b2])
                            p2n += 1

    nc.compile()
    return nc


# ---------------------------------------------------------------- entry
LAST_RESULTS = None
LAST_NC = None
LAST_INMAPS = None


def kernel(**inputs):
    import os
    from concourse.bass_utils import run_bass_kernel_spmd

    x = np.asarray(inputs["x"], dtype=np.float32)
    x_pad, x_bf, idx16_all, dstrel_all, coef_all, meta = _prep(
        x, inputs["edge_index"])
    wpack, vpack = _pack_weights({k: np.asarray(v, dtype=np.float32)
                                  for k, v in inputs.items() if k != "edge_index"})

    nc = _build(meta)

    brow = np.asarray(inputs["topo_b"], dtype=np.float32).reshape(1, D).astype(BF16)
    in_maps = []
    for c in range(NCORES):
        xT = np.zeros((D, PCP), dtype=BF16)
        xT[:, :PC] = x_pad[c * PC:(c + 1) * PC].T.astype(BF16)
        in_maps.append({
            "xbf": x_bf, "xT": xT,
            "idx16": idx16_all[c], "dstrel": dstrel_all[c], "coef": coef_all[c],
            "wpack": wpack, "vpack": vpack, "brow": brow,
        })
    global LAST_RESULTS, LAST_NC, LAST_INMAPS
    LAST_NC, LAST_INMAPS = nc, in_maps
    res = run_bass_kernel_spmd(nc, in_maps, core_ids=list(range(NCORES)),
                               trace=bool(os.environ.get("KTRACE")))
    LAST_RESULTS = res
    out = np.concatenate([res.results[c]["out"] for c in range(NCORES)], axis=0)
    return out[:N].astype(np.float32)


# revision 47
# speedup vs baseline: 1.9141x; 1.2742x over previous
"""EnhancedGTATLayer Trainium2 kernel — 8-core SPMD Bass implementation.

Host: sorts edges by (dst-group, src-half), pads to a uniform cross-core
chunk structure (one SPMD NEFF), packs int16 gather indices and per-slot
(one-hot dst, GCN-norm coefficient) pairs.  x is shipped twice: full copy
in bf16 (gather source) and a per-core feature-transposed slice in bf16.

Device (per core, 6272 dst nodes = 49 groups of 128), feature-transposed
[feat, node] layout, bf16 matmuls throughout (PSUM accumulate fp32):
  pass 1: dma_gather x rows by src (bf16, 256B rows); S[e,d] =
          (dstrel[e]==d)*norm[e] built on DVE in bf16 (4x mode);
          z^T += Xe^T S in PSUM; y0 = agg = gcn_W^T z + b and
          y1 = topo_W^T x + b persisted in bf16; LN stats (sum, sumsq)
          via one-hot selector matmuls into a [32,512] PSUM tile.
  interlude (per 7-chunk batch): R' = rsqrt(128*s2 - s1^2 + 16384*eps)
          (= rstd/128) via DVE Newton; Q[0:16]=s1, Q[16:32]=R' in bf16.
  pass 2: yps = I@y - mu (broadcast matmul accumulated in PSUM);
          ln = Act(yps * R'_bcast, Relu, scale=128*g, bias=beta);
          sigmoid gates; fused = Sum_h W1_h^T(gate_h*diff) + W1s^T topo;
          MLP, residual; PE-transpose to row layout, one DMA per chunk.
"""
import sys

sys.path.insert(0, "/opt/trn_rl_repo")

import os
import numpy as np
import ml_dtypes

BF16 = ml_dtypes.bfloat16
GMODE = os.environ.get("GMODE", "full")  # full | half | off (timing experiments)
SFRAC = int(os.environ.get("SFRAC", "0"))  # every SFRAC-th S-build on Pool (0=off)

N = 50000
NP = 50176          # padded to 392*128
PC = 6272           # nodes per core = 49*128
PCP = 6656          # padded to 13*512 for uniform 512-wide chunks
NCORES = 8
G = 49              # dst groups of 128 per core
D = 128             # feature dim (CIN == COUT)
H = 4
EPS = 1e-5
HALF = 32768        # int16 index split
GSEG = 1            # dst groups per gather segment
NSEG = 49           # one per group
BSEG = 4            # dst groups per chunk (512 cols)
NCHUNK = 13         # ceil(49/4); last chunk has 1 group
BATCHES = [int(x) for x in os.environ.get("BAT", "4,4,3,2").split(",")]
RSQRT_C = 0x5F3759DF
NEWTON_ITERS = 1


# ---------------------------------------------------------------- host prep
def _prep(x, edge_index):
    src = np.asarray(edge_index[0], dtype=np.int64)
    dst = np.asarray(edge_index[1], dtype=np.int64)
    loops = np.arange(NP, dtype=np.int64)
    src_all = np.concatenate([src, loops])
    dst_all = np.concatenate([dst, loops])

    deg = np.bincount(dst_all, minlength=NP)
    x_pad = np.zeros((NP, D), dtype=np.float32)
    x_pad[:N] = np.asarray(x, dtype=np.float32)

    core_of = dst_all // PC
    per_core = []
    counts = np.zeros((NCORES, G, 2), dtype=np.int64)
    for c in range(NCORES):
        m = core_of == c
        s = src_all[m]
        dl = dst_all[m] - c * PC
        g = dl >> 7
        h = (s >= HALF).astype(np.int64)
        order = np.lexsort((s, h, g))
        s, dl, h = s[order], dl[order], h[order]
        key = (dl >> 7) * 2 + h
        counts[c] = np.bincount(key, minlength=G * 2).reshape(G, 2)
        per_core.append((s, dl, key))

    cmax = ((counts + 127) // 128).max(axis=0)               # [G, 2] chunks
    cntmax = counts.max(axis=0)                              # [G, 2] rows
    ch_off = np.zeros((2, G), dtype=np.int64)
    ch_off[0] = np.concatenate([[0], np.cumsum(cmax[:, 0])[:-1]])
    nch_lo = int(cmax[:, 0].sum())
    ch_off[1] = nch_lo + np.concatenate([[0], np.cumsum(cmax[:, 1])[:-1]])
    totch = nch_lo + int(cmax[:, 1].sum())
    totslots = totch * 128

    dis = deg.astype(np.float32) ** -0.5   # reference: deg ** -0.5 in f32

    idx16_all, dstrel_all, coef_all = [], [], []
    for c in range(NCORES):
        s, dl, key = per_core[c]
        idx = np.full(totslots, -1, dtype=np.int16)
        dr = np.full(totslots, -1.0, dtype=np.float32)
        cf = np.zeros(totslots, dtype=np.float32)
        starts = np.concatenate([[0], np.cumsum(np.bincount(key, minlength=G * 2))])
        for g in range(G):
            for h in range(2):
                a, b = starts[g * 2 + h], starts[g * 2 + h + 1]
                off = ch_off[h, g] * 128
                # real edges, then dummy-valid rows up to the cross-core max
                # count (uniform num_idxs_reg), then -1 tail (not transferred)
                idx[off:off + (b - a)] = (s[a:b] - (HALF if h else 0)).astype(np.int16)
                idx[off + (b - a):off + int(cntmax[g, h])] = 0
                if b > a:
                    dr[off:off + (b - a)] = (dl[a:b] & 127).astype(np.float32)
                    cf[off:off + (b - a)] = dis[s[a:b]] * dis[dl[a:b] + c * PC]
        idx16_all.append(np.tile(idx.reshape(-1, 16).T, (8, 1)).astype(np.int16))
        dstrel_all.append(np.ascontiguousarray(dr.reshape(totch, 128).T))
        coef_all.append(np.ascontiguousarray(cf.reshape(totch, 128).T))

    meta = dict(cmax=cmax, ch_off=ch_off, totch=totch, cntmax=cntmax)
    x_bf = x_pad.astype(BF16)
    return x_pad, x_bf, idx16_all, dstrel_all, coef_all, meta


def _pack_weights(ins):
    w = np.zeros((17, D, D), dtype=np.float32)
    w[0] = ins["gcn_W"]
    w[1] = ins["topo_W"]
    w[2] = ins["res_W"]
    w[3] = ins["mlp_W2"]
    for h in range(H):
        w[4 + h] = ins["mlp_W1"][h * D:(h + 1) * D, :]
        w[8 + 2 * h] = ins["attn_W"][h][:D, :]
        w[9 + 2 * h] = ins["attn_W"][h][D:, :]
        w[16] += ins["mlp_W1"][h * D:(h + 1) * D, :]
    v = np.zeros((D, 13), dtype=np.float32)
    v[:, 0] = ins["gcn_b"]
    v[:, 1] = 128.0 * ins["ln_node_g"]
    v[:, 2] = ins["ln_node_b"]
    v[:, 3] = 128.0 * ins["ln_topo_g"]
    v[:, 4] = ins["ln_topo_b"]
    v[:, 5] = ins["topo_b"]
    for h in range(H):
        v[:, 6 + h] = ins["attn_b"][h]
    v[:, 10] = ins["mlp_b1"]
    v[:, 11] = ins["mlp_b2"]
    v[:, 12] = ins["res_b"]
    return w.astype(BF16), v


# ---------------------------------------------------------------- device
def _build(meta):
    import concourse.bacc as bacc
    import concourse.tile as tile
    from concourse import mybir
    from contextlib import ExitStack

    cmax, ch_off, totch = meta["cmax"], meta["ch_off"], meta["totch"]
    F32, BF, I16, I32 = (mybir.dt.float32, mybir.dt.bfloat16,
                         mybir.dt.int16, mybir.dt.int32)
    AF = mybir.ActivationFunctionType
    OP = mybir.AluOpType

    nc = bacc.Bacc("TRN2", target_bir_lowering=False, num_devices=NCORES,
                   dynamic_dma_scratch_size=65536, num_swdge_queues=4)
    t_xbf = nc.dram_tensor("xbf", [NP, D], BF, kind="ExternalInput")
    t_xT = nc.dram_tensor("xT", [D, PCP], BF, kind="ExternalInput")
    t_idx = nc.dram_tensor("idx16", [128, totch * 8], I16, kind="ExternalInput")
    t_dstrel = nc.dram_tensor("dstrel", [128, totch], F32, kind="ExternalInput")
    t_coef = nc.dram_tensor("coef", [128, totch], F32, kind="ExternalInput")
    t_wpack = nc.dram_tensor("wpack", [17, D, D], BF, kind="ExternalInput")
    t_vpack = nc.dram_tensor("vpack", [D, 13], F32, kind="ExternalInput")
    t_brow = nc.dram_tensor("brow", [1, D], BF, kind="ExternalInput")
    t_out = nc.dram_tensor("out", [PC, D], F32, kind="ExternalOutput")

    iota_np = np.broadcast_to(np.arange(128, dtype=np.float32),
                              (128, 128)).astype(BF16)
    t_iota = nc.inline_tensor(iota_np.copy(), name="iota128")
    t_id128 = nc.inline_tensor(np.eye(128, dtype=np.float32).astype(BF16),
                               name="ident128")
    t_id128f = nc.inline_tensor(np.eye(128, dtype=np.float32), name="ident128f")
    # stats selectors: [128, 16, 16], [:, k, j] = (j == k)
    idrep = np.broadcast_to(np.eye(16, dtype=np.float32), (128, 16, 16))
    t_idrep = nc.inline_tensor(idrep.astype(BF16).copy(), name="idrep16")
    # broadcast selectors over the [32, 512] Q tile (rows 0:16 = s1 sums,
    # rows 16:32 = R'):  bqone picks R' row, bqneg adds -s1/128 (= -mu).
    bqone = np.zeros((64, 12, 128), dtype=np.float32)
    bqneg = np.zeros((64, 12, 128), dtype=np.float32)
    for k in range(12):
        bqone[32 + k, k, :] = 1.0
        bqneg[k, k, :] = -1.0 / 128.0
    t_bqone = nc.inline_tensor(bqone.astype(BF16), name="bqone")
    t_bqneg = nc.inline_tensor(bqneg.astype(BF16), name="bqneg")

    # gather segment geometry (uniform across cores)
    cntmax = meta["cntmax"]
    seg_lo, seg_hi = [], []
    for s in range(NSEG):
        g0, g1 = s * GSEG, min((s + 1) * GSEG, G)
        seg_lo.append((int(ch_off[0, g0]), int(cmax[g0:g1, 0].sum()),
                       int(cntmax[g0:g1, 0].sum())))
        seg_hi.append((int(ch_off[1, g0]), int(cmax[g0:g1, 1].sum()),
                       int(cntmax[g0:g1, 1].sum())))
    max_lo = max(n for _, n, _ in seg_lo)
    max_hi = max(n for _, n, _ in seg_hi)

    with ExitStack() as ctx:
        tc = ctx.enter_context(tile.TileContext(nc))
        keep = ctx.enter_context(tc.tile_pool(name="keep", bufs=1))

        # ---------------- persistent tiles (gather-critical inputs first)
        idx_sb = keep.tile([128, totch * 8], I16)
        nc.sync.dma_start(out=idx_sb[:], in_=t_idx.ap())
        dstrel_sb = keep.tile([128, totch], F32)
        nc.sync.dma_start(out=dstrel_sb[:], in_=t_dstrel.ap())
        coef_sb = keep.tile([128, totch], F32)
        nc.sync.dma_start(out=coef_sb[:], in_=t_coef.ap())
        iota_t = keep.tile([128, 128], BF)
        nc.sync.dma_start(out=iota_t[:], in_=t_iota.ap())
        wbf = keep.tile([128, 17, D], BF)
        nc.sync.dma_start(out=wbf[:], in_=t_wpack.ap().rearrange("b k m -> k b m"))
        vp = keep.tile([128, 13], F32)
        nc.sync.dma_start(out=vp[:], in_=t_vpack.ap())
        id128 = keep.tile([128, 128], BF)
        nc.sync.dma_start(out=id128[:], in_=t_id128.ap())
        id128f = keep.tile([128, 128], F32)
        nc.sync.dma_start(out=id128f[:], in_=t_id128f.ap())
        idrep_t = keep.tile([128, 16, 16], BF)
        nc.sync.dma_start(out=idrep_t[:], in_=t_idrep.ap())
        bqone_t = keep.tile([64, 12, 128], BF)
        nc.sync.dma_start(out=bqone_t[:], in_=t_bqone.ap())
        bqneg_t = keep.tile([64, 12, 128], BF)
        nc.sync.dma_start(out=bqneg_t[:], in_=t_bqneg.ap())
        xTbf = keep.tile([128, PCP], BF)
        nc.sync.dma_start(out=xTbf[:], in_=t_xT.ap())

        y0_all = keep.tile([128, PCP], BF)
        brow = keep.tile([1, D], BF)
        nc.sync.dma_start(out=brow[:], in_=t_brow.ap())
        ones_row = keep.tile([1, 512], BF)
        nc.vector.memset(ones_row[:], 1.0)
        # Newton constants
        c_magic = keep.tile([16, 512], I32)
        nc.vector.memset(c_magic[:], RSQRT_C)
        c_one = keep.tile([16, 512], I32)
        nc.vector.memset(c_one[:], 1)

        def vcol(j):
            return vp[:, j:j + 1]

        with ExitStack() as pp:
            sb1 = pp.enter_context(tc.tile_pool(name="sb1", bufs=2))
            sb2 = pp.enter_context(tc.tile_pool(name="sb2", bufs=2))
            nwt = pp.enter_context(tc.tile_pool(name="nwt", bufs=1))
            qpl = pp.enter_context(tc.tile_pool(name="qpl", bufs=2))
            xel = pp.enter_context(tc.tile_pool(name="xel", bufs=10))
            xeh = pp.enter_context(tc.tile_pool(name="xeh", bufs=10))
            zpsp = pp.enter_context(tc.tile_pool(name="zpsp", bufs=2, space="PSUM"))
            stp = pp.enter_context(tc.tile_pool(name="stp", bufs=1, space="PSUM"))
            lnp = pp.enter_context(tc.tile_pool(name="lnp", bufs=2, space="PSUM"))
            bcp = pp.enter_context(tc.tile_pool(name="bcp", bufs=2, space="PSUM"))
            finp = pp.enter_context(tc.tile_pool(name="finp", bufs=1, space="PSUM"))

            gq = [0]  # emitted-gather counter: queue = gq % 4 keeps the
                      # round-robin DMA-sem assignment queue-consistent
            warmed = []
            for _w in range(10):
                xwl = xel.tile([128, max_lo, D], BF, tag="xel")
                nc.vector.memset(xwl[:], 0.0)
                xwh = xeh.tile([128, max_hi, D], BF, tag="xeh")
                nc.vector.memset(xwh[:], 0.0)
                warmed.append((xwl, xwh))

            def pass1_chunk(ci, st, cl, blen):
                g0, g1 = ci * BSEG, min((ci + 1) * BSEG, G)
                W = (g1 - g0) * 128        # scatter width (may be 128)
                n0 = g0 * 128
                last = cl == blen - 1
                segs = sorted({g // GSEG for g in range(g0, g1)})
                xe_map = {}
                for s in segs:
                    lo_start, lo_n, lo_cnt = seg_lo[s]
                    hi_start, hi_n, hi_cnt = seg_hi[s]
                    xe_lo = xel.tile([128, max_lo, D], BF, tag="xel")
                    xe_hi = xeh.tile([128, max_hi, D], BF, tag="xeh")
                    esz = D
                    if GMODE == "off":
                        nc.vector.memset(xe_lo[:], 0.0)
                        nc.vector.memset(xe_hi[:], 0.0)
                    if lo_n and lo_cnt and GMODE != "off":
                        nc.gpsimd.dma_gather(
                            out_ap=xe_lo[:, :lo_n, :esz],
                            in_ap=t_xbf.ap()[0:HALF, 0:esz],
                            idxs_ap=idx_sb[:, lo_start * 8:(lo_start + lo_n) * 8],
                            num_idxs=lo_n * 128, num_idxs_reg=lo_cnt,
                            elem_size=esz, elem_step=D, single_packet=False,
                            queue_num=gq[0] % 4)
                        gq[0] += 1
                    if hi_n and hi_cnt and GMODE != "off":
                        nc.gpsimd.dma_gather(
                            out_ap=xe_hi[:, :hi_n, :esz],
                            in_ap=t_xbf.ap()[HALF:NP, 0:esz],
                            idxs_ap=idx_sb[:, hi_start * 8:(hi_start + hi_n) * 8],
                            num_idxs=hi_n * 128, num_idxs_reg=hi_cnt,
                            elem_size=esz, elem_step=D, single_packet=False,
                            queue_num=gq[0] % 4)
                        gq[0] += 1
                    xe_map[s] = (xe_lo, xe_hi)

                zps = zpsp.tile([128, 512], F32, space="PSUM", tag="zps")
                if W < 512:
                    nc.vector.memset(zps[:, W:], 0.0)
                for g in range(g0, g1):
                    col = (g - g0) * 128
                    xe_lo, xe_hi = xe_map[g // GSEG]
                    mms = []
                    for h, (xe, stt) in enumerate(
                            [(xe_lo, seg_lo[g // GSEG][0]),
                             (xe_hi, seg_hi[g // GSEG][0])]):
                        for k in range(int(cmax[g, h])):
                            gch = int(ch_off[h, g]) + k
                            sch = gch - stt
                            s_t = sb1.tile([128, 128], BF, tag="s_t")
                            # S[e, d] = (dstrel[e] == d) * norm[e]
                            eng = (nc.gpsimd if SFRAC and (gch % SFRAC == 0)
                                   else nc.vector)
                            eng.tensor_scalar(
                                out=s_t[:], in0=iota_t[:],
                                scalar1=dstrel_sb[:, gch:gch + 1],
                                scalar2=coef_sb[:, gch:gch + 1],
                                op0=OP.is_equal, op1=OP.mult)
                            mms.append((xe, sch, s_t))
                    for mi, (xe, sch, s_t) in enumerate(mms):
                        nc.tensor.matmul(
                            out=zps[:, col:col + 128], lhsT=xe[:, sch, :],
                            rhs=s_t[:], start=(mi == 0), stop=(mi == len(mms) - 1))

                u_t = sb1.tile([128, 512], BF, tag="u")
                nc.scalar.copy(out=u_t[:], in_=zps[:])

                # agg = gcn_W^T z (+b);  topo = topo_W^T x (+b); both bf16
                aps = zpsp.tile([128, 512], F32, space="PSUM", tag="zps")
                nc.tensor.matmul(out=aps[:], lhsT=wbf[:, 0, :],
                                 rhs=u_t[:], start=True, stop=True)
                nc.scalar.activation(out=y0_all[:, n0:n0 + 512], in_=aps[:],
                                     func=AF.Identity, bias=vcol(0))
                sq = sb1.tile([128, 512], BF, tag="sq")
                nc.scalar.activation(out=sq[:], in_=aps[:],
                                     func=AF.Square, bias=vcol(0))
                nc.tensor.matmul(out=st[0:16, :], lhsT=idrep_t[:, cl, :],
                                 rhs=y0_all[:, n0:n0 + 512], start=(cl == 0),
                                 stop=False, skip_group_check=True)
                nc.tensor.matmul(out=st[32:48, :], lhsT=idrep_t[:, cl, :],
                                 rhs=sq[:], start=(cl == 0), stop=False,
                                 skip_group_check=True)

                tps = zpsp.tile([128, 512], F32, space="PSUM", tag="zps")
                nc.tensor.matmul(out=tps[:], lhsT=wbf[:, 1, :],
                                 rhs=xTbf[:, n0:n0 + 512], start=True, stop=True)
                y1 = sb1.tile([128, 512], BF, tag="y1")
                nc.scalar.activation(out=y1[:], in_=tps[:],
                                     func=AF.Identity, bias=vcol(5))
                sqt = sb1.tile([128, 512], BF, tag="sqt")
                nc.scalar.activation(out=sqt[:], in_=tps[:],
                                     func=AF.Square, bias=vcol(5))
                nc.tensor.matmul(out=st[0:16, :], lhsT=idrep_t[:, 8 + cl, :],
                                 rhs=y1[:], start=False,
                                 stop=last, skip_group_check=True)
                nc.tensor.matmul(out=st[32:48, :], lhsT=idrep_t[:, 8 + cl, :],
                                 rhs=sqt[:], start=False, stop=last,
                                 skip_group_check=True)

            def interlude(st):
                """Q[0:16] = s1 (bf16), Q[32:48] = R' = rsqrt(128*s2 - s1^2
                + 16384*eps) = rstd/128, via DVE Newton."""
                s1c_t = nwt.tile([16, 512], F32, tag="s1c")
                q_t = nwt.tile([16, 512], F32, tag="qq")
                v_t = nwt.tile([16, 512], F32, tag="vv")
                yi_t = nwt.tile([16, 512], I32, tag="yy")
                t_t = nwt.tile([16, 512], F32, tag="tt0")
                tt_t = nwt.tile([16, 512], F32, tag="tt1")
                s1c, q, v, t, tt = s1c_t[:], q_t[:], v_t[:], t_t[:], tt_t[:]
                yi = yi_t[:]
                nc.vector.tensor_copy(out=s1c, in_=st[0:16, :])
                nc.vector.tensor_scalar(out=t, in0=st[32:48, :],
                                        scalar1=128.0, scalar2=16384.0 * EPS,
                                        op0=OP.mult, op1=OP.add)
                nc.vector.tensor_tensor(out=q, in0=s1c, in1=s1c, op=OP.mult)
                nc.vector.tensor_tensor(out=v, in0=t, in1=q, op=OP.subtract)
                # Newton rsqrt: y0 = bits(C - (bits(v) >> 1))
                nc.vector.tensor_tensor(out=yi, in0=v.bitcast(I32),
                                        in1=c_one[:], op=OP.arith_shift_right)
                nc.vector.tensor_tensor(out=yi, in0=c_magic[:], in1=yi,
                                        op=OP.subtract)
                y = yi.bitcast(F32)
                for _ in range(NEWTON_ITERS):
                    nc.vector.tensor_tensor(out=tt, in0=y, in1=y, op=OP.mult)
                    nc.vector.tensor_tensor(out=tt, in0=tt, in1=v, op=OP.mult)
                    nc.vector.tensor_scalar(out=tt, in0=tt, scalar1=-0.5,
                                            scalar2=1.5, op0=OP.mult, op1=OP.add)
                    nc.vector.tensor_tensor(out=y, in0=y, in1=tt, op=OP.mult)
                qt = qpl.tile([64, 512], BF, tag="q")
                nc.vector.memset(qt[:], 0.0)
                nc.scalar.copy(out=qt[0:16, :], in_=s1c)
                nc.scalar.copy(out=qt[32:48, :], in_=yi.bitcast(F32))
                return qt

            def pass2_chunk(ci, cl, qt):
                g0, g1 = ci * BSEG, min((ci + 1) * BSEG, G)
                W = (g1 - g0) * 128
                n0 = g0 * 128

                def layernorm(yall, k, g_j, b_j, ln):
                    yps = lnp.tile([128, 512], F32, space="PSUM", tag="ln")
                    if yall is None:
                        # recompute topo = topo_W^T x + b (rank-1 bias bcast)
                        nc.tensor.matmul(out=yps[:], lhsT=wbf[:, 1, :],
                                         rhs=xTbf[:, n0:n0 + 512], start=True,
                                         stop=False)
                        nc.tensor.matmul(out=yps[:], lhsT=brow[:],
                                         rhs=ones_row[:], start=False,
                                         stop=False, skip_group_check=True)
                    else:
                        nc.tensor.matmul(out=yps[:], lhsT=id128[:],
                                         rhs=yall[:, n0:n0 + 512], start=True,
                                         stop=False)
                    nc.tensor.matmul(out=yps[:], lhsT=bqneg_t[:, k, :],
                                     rhs=qt[:], start=False, stop=True)
                    rb = bcp.tile([128, 512], F32, space="PSUM", tag="bc")
                    nc.tensor.matmul(out=rb[:], lhsT=bqone_t[:, k, :],
                                     rhs=qt[:], start=True, stop=True)
                    rbg = sb2.tile([128, 512], BF, tag="rbg" + str(ln))
                    nc.scalar.activation(out=rbg[:], in_=rb[:],
                                         func=AF.Identity, scale=vcol(g_j))
                    t1 = sb2.tile([128, 512], BF, tag="t1" + str(ln))
                    nc.vector.tensor_tensor(out=t1[:], in0=yps[:], in1=rbg[:],
                                            op=OP.mult)
                    o = sb2.tile([128, 512], BF, tag="nf" + str(ln))
                    nc.scalar.activation(out=o[:], in_=t1[:], func=AF.Relu,
                                         bias=vcol(b_j))
                    return o

                nf = layernorm(y0_all, cl, 1, 2, 0)
                tf = layernorm(None, 8 + cl, 3, 4, 1)

                diff = sb2.tile([128, 512], BF, tag="diff")
                nc.vector.tensor_tensor(out=diff[:], in0=nf[:], in1=tf[:],
                                        op=OP.subtract)

                h1ps = finp.tile([128, 512], F32, space="PSUM", tag="fin")
                nc.tensor.matmul(out=h1ps[:], lhsT=wbf[:, 16, :], rhs=tf[:],
                                 start=True, stop=False)
                for h in range(H):
                    gps = bcp.tile([128, 512], F32, space="PSUM", tag="bc")
                    nc.tensor.matmul(out=gps[:], lhsT=wbf[:, 8 + 2 * h, :],
                                     rhs=nf[:], start=True, stop=False)
                    nc.tensor.matmul(out=gps[:], lhsT=wbf[:, 9 + 2 * h, :],
                                     rhs=tf[:], start=False, stop=True)
                    gate = sb2.tile([128, 512], BF, tag="gate")
                    nc.scalar.activation(out=gate[:], in_=gps[:],
                                         func=AF.Sigmoid, bias=vcol(6 + h))
                    fh = sb2.tile([128, 512], BF, tag="fh")
                    nc.vector.tensor_tensor(out=fh[:], in0=gate[:],
                                            in1=diff[:], op=OP.mult)
                    nc.tensor.matmul(out=h1ps[:], lhsT=wbf[:, 4 + h, :],
                                     rhs=fh[:], start=False, stop=(h == H - 1))

                h1 = sb2.tile([128, 512], BF, tag="h1")
                nc.scalar.activation(out=h1[:], in_=h1ps[:],
                                     func=AF.Relu, bias=vcol(10))
                h2ps = finp.tile([128, 512], F32, space="PSUM", tag="fin")
                nc.tensor.matmul(out=h2ps[:], lhsT=wbf[:, 3, :],
                                 rhs=h1[:], start=True, stop=True)
                rps = lnp.tile([128, 512], F32, space="PSUM", tag="ln")
                nc.tensor.matmul(out=rps[:], lhsT=wbf[:, 2, :],
                                 rhs=xTbf[:, n0:n0 + 512], start=True, stop=True)
                rph = sb2.tile([128, 512], BF, tag="rph")
                nc.scalar.activation(out=rph[:], in_=rps[:],
                                     func=AF.Identity, bias=vcol(12))
                h2 = sb2.tile([128, 512], BF, tag="h2")
                nc.vector.tensor_scalar(out=h2[:], in0=h2ps[:],
                                        scalar1=vcol(11), scalar2=0.0,
                                        op0=OP.add, op1=OP.max)
                o = sb2.tile([128, 512], F32, tag="o")
                nc.vector.tensor_tensor(out=o[:], in0=h2[:], in1=rph[:],
                                        op=OP.add)

                ng = g1 - g0
                otp = finp.tile([128, 4, 128], F32, space="PSUM", tag="fin")
                for g in range(g0, g1):
                    col = (g - g0) * 128
                    nc.tensor.transpose(out=otp[:, g - g0, :],
                                        in_=o[:, col:col + 128],
                                        identity=id128f[:])
                orow = sb2.tile([128, 4, 128], F32, tag="orow")
                nc.scalar.copy(out=orow[:, :ng, :], in_=otp[:, :ng, :])
                nc.sync.dma_start(
                    out=t_out.ap()[n0:n0 + W, :].rearrange(
                        "(g p) d -> p g d", p=128),
                    in_=orow[:, :ng, :])

            starts = np.concatenate([[0], np.cumsum(BATCHES)]).astype(int)
            nb = len(BATCHES)
            qts = [None] * nb
            stv = [None] * nb
            for _b in range(nb):
                stt = stp.tile([48, 512], F32, space="PSUM", tag="st")
                stv[_b] = stt

            def stview(b):
                return stv[b]

            def batch_of(ci):
                return int(np.searchsorted(starts, ci, side="right") - 1)

            # greedy schedule: pass1 priority, interludes ASAP, pass2 fills in
            for ci in range(BATCHES[0]):
                pass1_chunk(ci, stview(0), ci, BATCHES[0])
            qts[0] = interlude(stview(0))
            p1n = starts[1]          # next pass1 chunk to emit
            p2n = 0                  # next pass2 chunk to emit
            while p1n < NCHUNK or p2n < NCHUNK:
                if p1n < NCHUNK:
                    b = batch_of(p1n)
                    pass1_chunk(p1n, stview(b), p1n - starts[b], BATCHES[b])
                    if p1n == starts[b + 1] - 1:
                        qts[b] = interlude(stview(b))
                    p1n += 1
                nemit = 1 + (p1n - p2n >= 5)
                for _ in range(nemit):
                    if p2n < NCHUNK:
                        b2 = batch_of(p2n)
                        if qts[b2] is not None:
                            pass2_chunk(p2n, p2n - starts[b2], qts[b2])
                            p2n += 1

    nc.compile()
    return nc


# ---------------------------------------------------------------- entry
LAST_RESULTS = None
LAST_NC = None
LAST_INMAPS = None


def kernel(**inputs):
    import os
    from concourse.bass_utils import run_bass_kernel_spmd

    x = np.asarray(inputs["x"], dtype=np.float32)
    x_pad, x_bf, idx16_all, dstrel_all, coef_all, meta = _prep(
        x, inputs["edge_index"])
    wpack, vpack = _pack_weights({k: np.asarray(v, dtype=np.float32)
                                  for k, v in inputs.items() if k != "edge_index"})

    nc = _build(meta)

    brow = np.asarray(inputs["topo_b"], dtype=np.float32).reshape(1, D).astype(BF16)
    in_maps = []
    for c in range(NCORES):
        xT = np.zeros((D, PCP), dtype=BF16)
        xT[:, :PC] = x_pad[c * PC:(c + 1) * PC].T.astype(BF16)
        in_maps.append({
            "xbf": x_bf, "xT": xT,
            "idx16": idx16_all[c], "dstrel": dstrel_all[c], "coef": coef_all[c],
            "wpack": wpack, "vpack": vpack, "brow": brow,
        })
    global LAST_RESULTS, LAST_NC, LAST_INMAPS
    LAST_NC, LAST_INMAPS = nc, in_maps
    res = run_bass_kernel_spmd(nc, in_maps, core_ids=list(range(NCORES)),
                               trace=bool(os.environ.get("KTRACE")))
    LAST_RESULTS = res
    out = np.concatenate([res.results[c]["out"] for c in range(NCORES)], axis=0)
    return out[:N].astype(np.float32)


# revision 51
# speedup vs baseline: 2.0134x; 1.0518x over previous
"""EnhancedGTATLayer Trainium2 kernel — 8-core SPMD Bass implementation.

Host: sorts edges by (dst-group, src-half), pads to a uniform cross-core
chunk structure (one SPMD NEFF), packs int16 gather indices and per-slot
(one-hot dst, GCN-norm coefficient) pairs.  x is shipped twice: full copy
in bf16 (gather source) and a per-core feature-transposed slice in bf16.

Device (per core, 6272 dst nodes = 49 groups of 128), feature-transposed
[feat, node] layout, bf16 matmuls throughout (PSUM accumulate fp32):
  pass 1: dma_gather x rows by src (bf16, 256B rows); S[e,d] =
          (dstrel[e]==d)*norm[e] built on DVE in bf16 (4x mode);
          z^T += Xe^T S in PSUM; y0 = agg = gcn_W^T z + b and
          y1 = topo_W^T x + b persisted in bf16; LN stats (sum, sumsq)
          via one-hot selector matmuls into a [32,512] PSUM tile.
  interlude (per 7-chunk batch): R' = rsqrt(128*s2 - s1^2 + 16384*eps)
          (= rstd/128) via DVE Newton; Q[0:16]=s1, Q[16:32]=R' in bf16.
  pass 2: yps = I@y - mu (broadcast matmul accumulated in PSUM);
          ln = Act(yps * R'_bcast, Relu, scale=128*g, bias=beta);
          sigmoid gates; fused = Sum_h W1_h^T(gate_h*diff) + W1s^T topo;
          MLP, residual; PE-transpose to row layout, one DMA per chunk.
"""
import sys

sys.path.insert(0, "/opt/trn_rl_repo")

import os
import numpy as np
import ml_dtypes

BF16 = ml_dtypes.bfloat16
GMODE = os.environ.get("GMODE", "full")  # full | half | off (timing experiments)
SFRAC = int(os.environ.get("SFRAC", "0"))  # every SFRAC-th S-build on Pool (0=off)
DRAIN = int(os.environ.get("DRAIN", "5"))  # extra-p2 backlog threshold
XEB = int(os.environ.get("XEB", "6"))      # gather buffer depth

N = 50000
NP = 50176          # padded to 392*128
PC = 6272           # nodes per core = 49*128
PCP = 6656          # padded to 13*512 for uniform 512-wide chunks
NCORES = 8
G = 49              # dst groups of 128 per core
D = 128             # feature dim (CIN == COUT)
H = 4
EPS = 1e-5
HALF = 32768        # int16 index split
GSEG = 1            # dst groups per gather segment
NSEG = 49           # one per group
BSEG = 4            # dst groups per chunk (512 cols)
NCHUNK = 13         # ceil(49/4); last chunk has 1 group
BATCHES = [int(x) for x in os.environ.get("BAT", "4,4,3,2").split(",")]
RSQRT_C = 0x5F3759DF
NEWTON_ITERS = 1


# ---------------------------------------------------------------- host prep
def _prep(x, edge_index):
    src = np.asarray(edge_index[0], dtype=np.int64)
    dst = np.asarray(edge_index[1], dtype=np.int64)
    loops = np.arange(NP, dtype=np.int64)
    src_all = np.concatenate([src, loops])
    dst_all = np.concatenate([dst, loops])

    deg = np.bincount(dst_all, minlength=NP)
    x_pad = np.zeros((NP, D), dtype=np.float32)
    x_pad[:N] = np.asarray(x, dtype=np.float32)

    core_of = dst_all // PC
    per_core = []
    counts = np.zeros((NCORES, G, 2), dtype=np.int64)
    for c in range(NCORES):
        m = core_of == c
        s = src_all[m]
        dl = dst_all[m] - c * PC
        g = dl >> 7
        h = (s >= HALF).astype(np.int64)
        order = np.lexsort((s, h, g))
        s, dl, h = s[order], dl[order], h[order]
        key = (dl >> 7) * 2 + h
        counts[c] = np.bincount(key, minlength=G * 2).reshape(G, 2)
        per_core.append((s, dl, key))

    cmax = ((counts + 127) // 128).max(axis=0)               # [G, 2] chunks
    cntmax = counts.max(axis=0)                              # [G, 2] rows
    ch_off = np.zeros((2, G), dtype=np.int64)
    ch_off[0] = np.concatenate([[0], np.cumsum(cmax[:, 0])[:-1]])
    nch_lo = int(cmax[:, 0].sum())
    ch_off[1] = nch_lo + np.concatenate([[0], np.cumsum(cmax[:, 1])[:-1]])
    totch = nch_lo + int(cmax[:, 1].sum())
    totslots = totch * 128

    dis = deg.astype(np.float32) ** -0.5   # reference: deg ** -0.5 in f32

    idx16_all, dstrel_all, coef_all = [], [], []
    for c in range(NCORES):
        s, dl, key = per_core[c]
        idx = np.full(totslots, -1, dtype=np.int16)
        dr = np.full(totslots, -1.0, dtype=np.float32)
        cf = np.zeros(totslots, dtype=np.float32)
        starts = np.concatenate([[0], np.cumsum(np.bincount(key, minlength=G * 2))])
        for g in range(G):
            for h in range(2):
                a, b = starts[g * 2 + h], starts[g * 2 + h + 1]
                off = ch_off[h, g] * 128
                # real edges, then dummy-valid rows up to the cross-core max
                # count (uniform num_idxs_reg), then -1 tail (not transferred)
                idx[off:off + (b - a)] = (s[a:b] - (HALF if h else 0)).astype(np.int16)
                idx[off + (b - a):off + int(cntmax[g, h])] = 0
                if b > a:
                    dr[off:off + (b - a)] = (dl[a:b] & 127).astype(np.float32)
                    cf[off:off + (b - a)] = dis[s[a:b]] * dis[dl[a:b] + c * PC]
        idx16_all.append(np.tile(idx.reshape(-1, 16).T, (8, 1)).astype(np.int16))
        dstrel_all.append(np.ascontiguousarray(dr.reshape(totch, 128).T))
        coef_all.append(np.ascontiguousarray(cf.reshape(totch, 128).T))

    meta = dict(cmax=cmax, ch_off=ch_off, totch=totch, cntmax=cntmax)
    x_bf = x_pad.astype(BF16)
    return x_pad, x_bf, idx16_all, dstrel_all, coef_all, meta


def _pack_weights(ins):
    w = np.zeros((17, D, D), dtype=np.float32)
    w[0] = ins["gcn_W"]
    w[1] = ins["topo_W"]
    w[2] = ins["res_W"]
    w[3] = ins["mlp_W2"]
    for h in range(H):
        w[4 + h] = ins["mlp_W1"][h * D:(h + 1) * D, :]
        w[8 + 2 * h] = ins["attn_W"][h][:D, :]
        w[9 + 2 * h] = ins["attn_W"][h][D:, :]
        w[16] += ins["mlp_W1"][h * D:(h + 1) * D, :]
    v = np.zeros((D, 13), dtype=np.float32)
    v[:, 0] = ins["gcn_b"]
    v[:, 1] = 128.0 * ins["ln_node_g"]
    v[:, 2] = ins["ln_node_b"]
    v[:, 3] = 128.0 * ins["ln_topo_g"]
    v[:, 4] = ins["ln_topo_b"]
    v[:, 5] = ins["topo_b"]
    for h in range(H):
        v[:, 6 + h] = ins["attn_b"][h]
    v[:, 10] = ins["mlp_b1"]
    v[:, 11] = ins["mlp_b2"]
    v[:, 12] = ins["res_b"]
    return w.astype(BF16), v


# ---------------------------------------------------------------- device
def _build(meta):
    import concourse.bacc as bacc
    import concourse.tile as tile
    from concourse import mybir
    from contextlib import ExitStack

    cmax, ch_off, totch = meta["cmax"], meta["ch_off"], meta["totch"]
    F32, BF, I16, I32 = (mybir.dt.float32, mybir.dt.bfloat16,
                         mybir.dt.int16, mybir.dt.int32)
    AF = mybir.ActivationFunctionType
    OP = mybir.AluOpType

    nc = bacc.Bacc("TRN2", target_bir_lowering=False, num_devices=NCORES,
                   dynamic_dma_scratch_size=65536, num_swdge_queues=4)
    t_xbf = nc.dram_tensor("xbf", [NP, D], BF, kind="ExternalInput")
    t_xT = nc.dram_tensor("xT", [D, PCP], BF, kind="ExternalInput")
    t_idx = nc.dram_tensor("idx16", [128, totch * 8], I16, kind="ExternalInput")
    t_dstrel = nc.dram_tensor("dstrel", [128, totch], F32, kind="ExternalInput")
    t_coef = nc.dram_tensor("coef", [128, totch], F32, kind="ExternalInput")
    t_wpack = nc.dram_tensor("wpack", [17, D, D], BF, kind="ExternalInput")
    t_vpack = nc.dram_tensor("vpack", [D, 13], F32, kind="ExternalInput")
    t_brow = nc.dram_tensor("brow", [1, D], BF, kind="ExternalInput")
    t_out = nc.dram_tensor("out", [PC, D], F32, kind="ExternalOutput")

    iota_np = np.broadcast_to(np.arange(128, dtype=np.float32),
                              (128, 128)).astype(BF16)
    t_iota = nc.inline_tensor(iota_np.copy(), name="iota128")
    t_id128 = nc.inline_tensor(np.eye(128, dtype=np.float32).astype(BF16),
                               name="ident128")
    t_id128f = nc.inline_tensor(np.eye(128, dtype=np.float32), name="ident128f")
    # stats selectors: [128, 16, 16], [:, k, j] = (j == k)
    idrep = np.broadcast_to(np.eye(16, dtype=np.float32), (128, 16, 16))
    t_idrep = nc.inline_tensor(idrep.astype(BF16).copy(), name="idrep16")
    # broadcast selectors over the [32, 512] Q tile (rows 0:16 = s1 sums,
    # rows 16:32 = R'):  bqone picks R' row, bqneg adds -s1/128 (= -mu).
    bqone = np.zeros((64, 16, 128), dtype=np.float32)
    bqneg = np.zeros((64, 16, 128), dtype=np.float32)
    for k in range(16):
        bqone[32 + k, k, :] = 1.0
        bqneg[k, k, :] = -1.0 / 128.0
    t_bqone = nc.inline_tensor(bqone.astype(BF16), name="bqone")
    t_bqneg = nc.inline_tensor(bqneg.astype(BF16), name="bqneg")

    # gather segment geometry (uniform across cores)
    cntmax = meta["cntmax"]
    seg_lo, seg_hi = [], []
    for s in range(NSEG):
        g0, g1 = s * GSEG, min((s + 1) * GSEG, G)
        seg_lo.append((int(ch_off[0, g0]), int(cmax[g0:g1, 0].sum()),
                       int(cntmax[g0:g1, 0].sum())))
        seg_hi.append((int(ch_off[1, g0]), int(cmax[g0:g1, 1].sum()),
                       int(cntmax[g0:g1, 1].sum())))
    max_lo = max(n for _, n, _ in seg_lo)
    max_hi = max(n for _, n, _ in seg_hi)

    with ExitStack() as ctx:
        tc = ctx.enter_context(tile.TileContext(nc))
        keep = ctx.enter_context(tc.tile_pool(name="keep", bufs=1))

        # ---------------- persistent tiles (gather-critical inputs first)
        idx_sb = keep.tile([128, totch * 8], I16)
        nc.sync.dma_start(out=idx_sb[:], in_=t_idx.ap())
        dstrel_sb = keep.tile([128, totch], F32)
        nc.sync.dma_start(out=dstrel_sb[:], in_=t_dstrel.ap())
        coef_sb = keep.tile([128, totch], F32)
        nc.sync.dma_start(out=coef_sb[:], in_=t_coef.ap())
        iota_t = keep.tile([128, 128], BF)
        nc.sync.dma_start(out=iota_t[:], in_=t_iota.ap())
        wbf = keep.tile([128, 17, D], BF)
        nc.sync.dma_start(out=wbf[:], in_=t_wpack.ap().rearrange("b k m -> k b m"))
        vp = keep.tile([128, 13], F32)
        nc.sync.dma_start(out=vp[:], in_=t_vpack.ap())
        id128 = keep.tile([128, 128], BF)
        nc.sync.dma_start(out=id128[:], in_=t_id128.ap())
        id128f = keep.tile([128, 128], F32)
        nc.sync.dma_start(out=id128f[:], in_=t_id128f.ap())
        idrep_t = keep.tile([128, 16, 16], BF)
        nc.sync.dma_start(out=idrep_t[:], in_=t_idrep.ap())
        bqone_t = keep.tile([64, 16, 128], BF)
        nc.sync.dma_start(out=bqone_t[:], in_=t_bqone.ap())
        bqneg_t = keep.tile([64, 16, 128], BF)
        nc.sync.dma_start(out=bqneg_t[:], in_=t_bqneg.ap())
        xTbf = keep.tile([128, PCP], BF)
        nc.sync.dma_start(out=xTbf[:], in_=t_xT.ap())

        y0_all = keep.tile([128, PCP], BF)
        brow = keep.tile([1, D], BF)
        nc.sync.dma_start(out=brow[:], in_=t_brow.ap())
        ones_row = keep.tile([1, 512], BF)
        nc.vector.memset(ones_row[:], 1.0)
        # Newton constants
        c_magic = keep.tile([16, 512], I32)
        nc.vector.memset(c_magic[:], RSQRT_C)
        c_one = keep.tile([16, 512], I32)
        nc.vector.memset(c_one[:], 1)

        def vcol(j):
            return vp[:, j:j + 1]

        with ExitStack() as pp:
            sb1 = pp.enter_context(tc.tile_pool(name="sb1", bufs=2))
            sb2 = pp.enter_context(tc.tile_pool(name="sb2", bufs=2))
            nwt = pp.enter_context(tc.tile_pool(name="nwt", bufs=1))
            qpl = pp.enter_context(tc.tile_pool(name="qpl", bufs=2))
            xel = pp.enter_context(tc.tile_pool(name="xel", bufs=XEB))
            xeh = pp.enter_context(tc.tile_pool(name="xeh", bufs=XEB))
            zpsp = pp.enter_context(tc.tile_pool(name="zpsp", bufs=2, space="PSUM"))
            stp = pp.enter_context(tc.tile_pool(name="stp", bufs=1, space="PSUM"))
            lnp = pp.enter_context(tc.tile_pool(name="lnp", bufs=2, space="PSUM"))
            bcp = pp.enter_context(tc.tile_pool(name="bcp", bufs=2, space="PSUM"))
            finp = pp.enter_context(tc.tile_pool(name="finp", bufs=1, space="PSUM"))

            gq = [0]  # emitted-gather counter: queue = gq % 4 keeps the
                      # round-robin DMA-sem assignment queue-consistent
            warmed = []
            for _w in range(XEB):
                xwl = xel.tile([128, max_lo, D], BF, tag="xel")
                nc.vector.memset(xwl[:], 0.0)
                xwh = xeh.tile([128, max_hi, D], BF, tag="xeh")
                nc.vector.memset(xwh[:], 0.0)
                warmed.append((xwl, xwh))

            def pass1_chunk(ci, st, cl, blen):
                g0, g1 = ci * BSEG, min((ci + 1) * BSEG, G)
                W = (g1 - g0) * 128        # scatter width (may be 128)
                n0 = g0 * 128
                last = cl == blen - 1
                segs = sorted({g // GSEG for g in range(g0, g1)})
                xe_map = {}
                for s in segs:
                    lo_start, lo_n, lo_cnt = seg_lo[s]
                    hi_start, hi_n, hi_cnt = seg_hi[s]
                    xe_lo = xel.tile([128, max_lo, D], BF, tag="xel")
                    xe_hi = xeh.tile([128, max_hi, D], BF, tag="xeh")
                    esz = D
                    if GMODE == "off":
                        nc.vector.memset(xe_lo[:], 0.0)
                        nc.vector.memset(xe_hi[:], 0.0)
                    if lo_n and lo_cnt and GMODE != "off":
                        nc.gpsimd.dma_gather(
                            out_ap=xe_lo[:, :lo_n, :esz],
                            in_ap=t_xbf.ap()[0:HALF, 0:esz],
                            idxs_ap=idx_sb[:, lo_start * 8:(lo_start + lo_n) * 8],
                            num_idxs=lo_n * 128, num_idxs_reg=lo_cnt,
                            elem_size=esz, elem_step=D, single_packet=False,
                            queue_num=gq[0] % 4)
                        gq[0] += 1
                    if hi_n and hi_cnt and GMODE != "off":
                        nc.gpsimd.dma_gather(
                            out_ap=xe_hi[:, :hi_n, :esz],
                            in_ap=t_xbf.ap()[HALF:NP, 0:esz],
                            idxs_ap=idx_sb[:, hi_start * 8:(hi_start + hi_n) * 8],
                            num_idxs=hi_n * 128, num_idxs_reg=hi_cnt,
                            elem_size=esz, elem_step=D, single_packet=False,
                            queue_num=gq[0] % 4)
                        gq[0] += 1
                    xe_map[s] = (xe_lo, xe_hi)

                zps = zpsp.tile([128, 512], F32, space="PSUM", tag="zps")
                if W < 512:
                    nc.vector.memset(zps[:, W:], 0.0)
                for g in range(g0, g1):
                    col = (g - g0) * 128
                    xe_lo, xe_hi = xe_map[g // GSEG]
                    mms = []
                    for h, (xe, stt) in enumerate(
                            [(xe_lo, seg_lo[g // GSEG][0]),
                             (xe_hi, seg_hi[g // GSEG][0])]):
                        for k in range(int(cmax[g, h])):
                            gch = int(ch_off[h, g]) + k
                            sch = gch - stt
                            s_t = sb1.tile([128, 128], BF, tag="s_t")
                            # S[e, d] = (dstrel[e] == d) * norm[e]
                            eng = (nc.gpsimd if SFRAC and (gch % SFRAC == 0)
                                   else nc.vector)
                            eng.tensor_scalar(
                                out=s_t[:], in0=iota_t[:],
                                scalar1=dstrel_sb[:, gch:gch + 1],
                                scalar2=coef_sb[:, gch:gch + 1],
                                op0=OP.is_equal, op1=OP.mult)
                            mms.append((xe, sch, s_t))
                    for mi, (xe, sch, s_t) in enumerate(mms):
                        nc.tensor.matmul(
                            out=zps[:, col:col + 128], lhsT=xe[:, sch, :],
                            rhs=s_t[:], start=(mi == 0), stop=(mi == len(mms) - 1))

                u_t = sb1.tile([128, 512], BF, tag="u")
                nc.scalar.copy(out=u_t[:], in_=zps[:])

                # agg = gcn_W^T z (+b);  topo = topo_W^T x (+b); both bf16
                aps = zpsp.tile([128, 512], F32, space="PSUM", tag="zps")
                nc.tensor.matmul(out=aps[:], lhsT=wbf[:, 0, :],
                                 rhs=u_t[:], start=True, stop=True)
                nc.scalar.activation(out=y0_all[:, n0:n0 + 512], in_=aps[:],
                                     func=AF.Identity, bias=vcol(0))
                sq = sb1.tile([128, 512], BF, tag="sq")
                nc.scalar.activation(out=sq[:], in_=aps[:],
                                     func=AF.Square, bias=vcol(0))
                nc.tensor.matmul(out=st[0:16, :], lhsT=idrep_t[:, cl, :],
                                 rhs=y0_all[:, n0:n0 + 512], start=(cl == 0),
                                 stop=False, skip_group_check=True)
                nc.tensor.matmul(out=st[32:48, :], lhsT=idrep_t[:, cl, :],
                                 rhs=sq[:], start=(cl == 0), stop=False,
                                 skip_group_check=True)

                tps = zpsp.tile([128, 512], F32, space="PSUM", tag="zps")
                nc.tensor.matmul(out=tps[:], lhsT=wbf[:, 1, :],
                                 rhs=xTbf[:, n0:n0 + 512], start=True, stop=True)
                y1 = sb1.tile([128, 512], BF, tag="y1")
                nc.scalar.activation(out=y1[:], in_=tps[:],
                                     func=AF.Identity, bias=vcol(5))
                sqt = sb1.tile([128, 512], BF, tag="sqt")
                nc.scalar.activation(out=sqt[:], in_=tps[:],
                                     func=AF.Square, bias=vcol(5))
                nc.tensor.matmul(out=st[0:16, :], lhsT=idrep_t[:, 8 + cl, :],
                                 rhs=y1[:], start=False,
                                 stop=last, skip_group_check=True)
                nc.tensor.matmul(out=st[32:48, :], lhsT=idrep_t[:, 8 + cl, :],
                                 rhs=sqt[:], start=False, stop=last,
                                 skip_group_check=True)

            def interlude(st):
                """Q[0:16] = s1 (bf16), Q[32:48] = R' = rsqrt(128*s2 - s1^2
                + 16384*eps) = rstd/128, via DVE Newton."""
                s1c_t = nwt.tile([16, 512], F32, tag="s1c")
                q_t = nwt.tile([16, 512], F32, tag="qq")
                v_t = nwt.tile([16, 512], F32, tag="vv")
                yi_t = nwt.tile([16, 512], I32, tag="yy")
                t_t = nwt.tile([16, 512], F32, tag="tt0")
                tt_t = nwt.tile([16, 512], F32, tag="tt1")
                s1c, q, v, t, tt = s1c_t[:], q_t[:], v_t[:], t_t[:], tt_t[:]
                yi = yi_t[:]
                nc.vector.tensor_copy(out=s1c, in_=st[0:16, :])
                nc.vector.tensor_scalar(out=t, in0=st[32:48, :],
                                        scalar1=128.0, scalar2=16384.0 * EPS,
                                        op0=OP.mult, op1=OP.add)
                nc.vector.tensor_tensor(out=q, in0=s1c, in1=s1c, op=OP.mult)
                nc.vector.tensor_tensor(out=v, in0=t, in1=q, op=OP.subtract)
                # Newton rsqrt: y0 = bits(C - (bits(v) >> 1))
                nc.vector.tensor_tensor(out=yi, in0=v.bitcast(I32),
                                        in1=c_one[:], op=OP.arith_shift_right)
                nc.vector.tensor_tensor(out=yi, in0=c_magic[:], in1=yi,
                                        op=OP.subtract)
                y = yi.bitcast(F32)
                for _ in range(NEWTON_ITERS):
                    nc.vector.tensor_tensor(out=tt, in0=y, in1=y, op=OP.mult)
                    nc.vector.tensor_tensor(out=tt, in0=tt, in1=v, op=OP.mult)
                    nc.vector.tensor_scalar(out=tt, in0=tt, scalar1=-0.5,
                                            scalar2=1.5, op0=OP.mult, op1=OP.add)
                    nc.vector.tensor_tensor(out=y, in0=y, in1=tt, op=OP.mult)
                qt = qpl.tile([64, 512], BF, tag="q")
                nc.vector.memset(qt[:], 0.0)
                nc.scalar.copy(out=qt[0:16, :], in_=s1c)
                nc.scalar.copy(out=qt[32:48, :], in_=yi.bitcast(F32))
                return qt

            def pass2_chunk(ci, cl, qt):
                g0, g1 = ci * BSEG, min((ci + 1) * BSEG, G)
                W = (g1 - g0) * 128
                n0 = g0 * 128

                def layernorm(yall, k, g_j, b_j, ln):
                    yps = lnp.tile([128, 512], F32, space="PSUM", tag="ln")
                    if yall is None:
                        # recompute topo = topo_W^T x + b (rank-1 bias bcast)
                        nc.tensor.matmul(out=yps[:], lhsT=wbf[:, 1, :],
                                         rhs=xTbf[:, n0:n0 + 512], start=True,
                                         stop=False)
                        nc.tensor.matmul(out=yps[:], lhsT=brow[:],
                                         rhs=ones_row[:], start=False,
                                         stop=False, skip_group_check=True)
                    else:
                        nc.tensor.matmul(out=yps[:], lhsT=id128[:],
                                         rhs=yall[:, n0:n0 + 512], start=True,
                                         stop=False)
                    nc.tensor.matmul(out=yps[:], lhsT=bqneg_t[:, k, :],
                                     rhs=qt[:], start=False, stop=True)
                    rb = bcp.tile([128, 512], F32, space="PSUM", tag="bc")
                    nc.tensor.matmul(out=rb[:], lhsT=bqone_t[:, k, :],
                                     rhs=qt[:], start=True, stop=True)
                    rbg = sb2.tile([128, 512], BF, tag="rbg" + str(ln))
                    nc.scalar.activation(out=rbg[:], in_=rb[:],
                                         func=AF.Identity, scale=vcol(g_j))
                    t1 = sb2.tile([128, 512], BF, tag="t1" + str(ln))
                    nc.vector.tensor_tensor(out=t1[:], in0=yps[:], in1=rbg[:],
                                            op=OP.mult)
                    o = sb2.tile([128, 512], BF, tag="nf" + str(ln))
                    nc.scalar.activation(out=o[:], in_=t1[:], func=AF.Relu,
                                         bias=vcol(b_j))
                    return o

                nf = layernorm(y0_all, cl, 1, 2, 0)
                tf = layernorm(None, 8 + cl, 3, 4, 1)

                diff = sb2.tile([128, 512], BF, tag="diff")
                nc.vector.tensor_tensor(out=diff[:], in0=nf[:], in1=tf[:],
                                        op=OP.subtract)

                h1ps = finp.tile([128, 512], F32, space="PSUM", tag="fin")
                nc.tensor.matmul(out=h1ps[:], lhsT=wbf[:, 16, :], rhs=tf[:],
                                 start=True, stop=False)
                for h in range(H):
                    gps = bcp.tile([128, 512], F32, space="PSUM", tag="bc")
                    nc.tensor.matmul(out=gps[:], lhsT=wbf[:, 8 + 2 * h, :],
                                     rhs=nf[:], start=True, stop=False)
                    nc.tensor.matmul(out=gps[:], lhsT=wbf[:, 9 + 2 * h, :],
                                     rhs=tf[:], start=False, stop=True)
                    gate = sb2.tile([128, 512], BF, tag="gate")
                    nc.scalar.activation(out=gate[:], in_=gps[:],
                                         func=AF.Sigmoid, bias=vcol(6 + h))
                    fh = sb2.tile([128, 512], BF, tag="fh")
                    nc.vector.tensor_tensor(out=fh[:], in0=gate[:],
                                            in1=diff[:], op=OP.mult)
                    nc.tensor.matmul(out=h1ps[:], lhsT=wbf[:, 4 + h, :],
                                     rhs=fh[:], start=False, stop=(h == H - 1))

                h1 = sb2.tile([128, 512], BF, tag="h1")
                nc.scalar.activation(out=h1[:], in_=h1ps[:],
                                     func=AF.Relu, bias=vcol(10))
                h2ps = finp.tile([128, 512], F32, space="PSUM", tag="fin")
                nc.tensor.matmul(out=h2ps[:], lhsT=wbf[:, 3, :],
                                 rhs=h1[:], start=True, stop=True)
                rps = lnp.tile([128, 512], F32, space="PSUM", tag="ln")
                nc.tensor.matmul(out=rps[:], lhsT=wbf[:, 2, :],
                                 rhs=xTbf[:, n0:n0 + 512], start=True, stop=True)
                rph = sb2.tile([128, 512], BF, tag="rph")
                nc.scalar.activation(out=rph[:], in_=rps[:],
                                     func=AF.Identity, bias=vcol(12))
                h2 = sb2.tile([128, 512], BF, tag="h2")
                nc.vector.tensor_scalar(out=h2[:], in0=h2ps[:],
                                        scalar1=vcol(11), scalar2=0.0,
                                        op0=OP.add, op1=OP.max)
                o = sb2.tile([128, 512], F32, tag="o")
                nc.vector.tensor_tensor(out=o[:], in0=h2[:], in1=rph[:],
                                        op=OP.add)

                ng = g1 - g0
                otp = finp.tile([128, 4, 128], F32, space="PSUM", tag="fin")
                for g in range(g0, g1):
                    col = (g - g0) * 128
                    nc.tensor.transpose(out=otp[:, g - g0, :],
                                        in_=o[:, col:col + 128],
                                        identity=id128f[:])
                orow = sb2.tile([128, 4, 128], F32, tag="orow")
                nc.scalar.copy(out=orow[:, :ng, :], in_=otp[:, :ng, :])
                nc.sync.dma_start(
                    out=t_out.ap()[n0:n0 + W, :].rearrange(
                        "(g p) d -> p g d", p=128),
                    in_=orow[:, :ng, :])

            starts = np.concatenate([[0], np.cumsum(BATCHES)]).astype(int)
            nb = len(BATCHES)
            qts = [None] * nb
            stv = [None] * nb
            for _b in range(nb):
                stt = stp.tile([48, 512], F32, space="PSUM", tag="st")
                stv[_b] = stt

            def stview(b):
                return stv[b]

            def batch_of(ci):
                return int(np.searchsorted(starts, ci, side="right") - 1)

            # greedy schedule: pass1 priority, interludes ASAP, pass2 fills in
            for ci in range(BATCHES[0]):
                pass1_chunk(ci, stview(0), ci, BATCHES[0])
            qts[0] = interlude(stview(0))
            p1n = starts[1]          # next pass1 chunk to emit
            p2n = 0                  # next pass2 chunk to emit
            while p1n < NCHUNK or p2n < NCHUNK:
                if p1n < NCHUNK:
                    b = batch_of(p1n)
                    pass1_chunk(p1n, stview(b), p1n - starts[b], BATCHES[b])
                    if p1n == starts[b + 1] - 1:
                        qts[b] = interlude(stview(b))
                    p1n += 1
                nemit = 1 + (p1n - p2n >= DRAIN)
                for _ in range(nemit):
                    if p2n < NCHUNK:
                        b2 = batch_of(p2n)
                        if qts[b2] is not None:
                            pass2_chunk(p2n, p2n - starts[b2], qts[b2])
                            p2n += 1

    nc.compile()
    return nc


# ---------------------------------------------------------------- entry
LAST_RESULTS = None
LAST_NC = None
LAST_INMAPS = None


def kernel(**inputs):
    import os
    from concourse.bass_utils import run_bass_kernel_spmd

    x = np.asarray(inputs["x"], dtype=np.float32)
    x_pad, x_bf, idx16_all, dstrel_all, coef_all, meta = _prep(
        x, inputs["edge_index"])
    wpack, vpack = _pack_weights({k: np.asarray(v, dtype=np.float32)
                                  for k, v in inputs.items() if k != "edge_index"})

    nc = _build(meta)

    brow = np.asarray(inputs["topo_b"], dtype=np.float32).reshape(1, D).astype(BF16)
    in_maps = []
    for c in range(NCORES):
        xT = np.zeros((D, PCP), dtype=BF16)
        xT[:, :PC] = x_pad[c * PC:(c + 1) * PC].T.astype(BF16)
        in_maps.append({
            "xbf": x_bf, "xT": xT,
            "idx16": idx16_all[c], "dstrel": dstrel_all[c], "coef": coef_all[c],
            "wpack": wpack, "vpack": vpack, "brow": brow,
        })
    global LAST_RESULTS, LAST_NC, LAST_INMAPS
    LAST_NC, LAST_INMAPS = nc, in_maps
    res = run_bass_kernel_spmd(nc, in_maps, core_ids=list(range(NCORES)),
                               trace=bool(os.environ.get("KTRACE")))
    LAST_RESULTS = res
    out = np.concatenate([res.results[c]["out"] for c in range(NCORES)], axis=0)
    return out[:N].astype(np.float32)
